# revision 1
# baseline (speedup 1.0000x reference)
"""Multi-head attention Trainium2 kernel (8 NeuronCores, SPMD).

Problem: B=4, S=2048, D_MODEL=1024, H=16, DIM=64 (nn_MultiHeadAttn).
Sharding: core c handles (batch b = c//2, query-row chunk c%2 of 1024).
Each core computes all 16 heads for its 1024 query rows against the full
2048 keys of its batch, then its rows of the output projection.

Device dataflow (host pre-transposes everything; device never transposes):
  - q^T/k^T/v^T arrive as [d_model, seq] f32 (tagged float32r so the PE
    runs 1 cycle/row) so the PE contracts over the partition dim directly.
  - Per-head q/k projections via block-diagonal [128,128] weights: one
    matmul pass projects a pair of heads.  v is projected into natural
    [s, e] layout with an extra all-ones column appended per head.
  - scores^T[k,q] = (kh^T chunk).T @ qh^T; exp with the 1/8 scale folded
    in (no max subtraction: |scores| < ~20 pre-scale, exp(x/8) <= e^2.5,
    and softmax is shift-invariant).  Exp work is split between ScalarE
    (exact spline exp) and VectorE (custom 8-stage DVE op evaluating a
    degree-3 polynomial p(x) ~= exp(x/32), then squaring twice).
  - out_h^T[e,q] (+ sum of exp in row 64) = (vh | ones).T @ attn^T
    accumulated over key chunks in PSUM.
  - normalize via fast reciprocal + partition-broadcast + multiply
    (partition moves via DMA since DVE lanes are partition-aligned).
  - out^T[o,q] = Wo^T-tiles.T @ hidden^T accumulated over e-tiles.
"""

import sys

if "/opt/trn_rl_repo" not in sys.path:
    sys.path.insert(0, "/opt/trn_rl_repo")

import numpy as np
from contextlib import ExitStack

N_CORES = 8
B, S, D = 4, 2048, 1024
H, DIM = 16, 64
SQ = 1024          # query rows per core
NPAIR = 8          # head pairs
NKC = S // 128     # key chunks of 128
VAW = 130          # width of augmented V-projection weights (2*(64+1))

# deg-3 minimax fit of exp(x/32) on |x|<=20; kernel computes p(x)^4=exp(x/8).
EXPC3 = 4.98779571e-06
EXPC2 = 5.03750782e-04
EXPC1 = 3.13034249e-02
EXPC0 = 9.99313241e-01

_cache = {}


def _register_exp_op():
    """Register the custom DVE exp op (deg-3 Horner + 2 squarings, 8 ALU
    stages) in concourse's custom-DVE registry; the per-NEFF uop table is
    generated from dve_ops.OPS at compile time."""
    if "exp_op" in _cache:
        return _cache["exp_op"]
    from concourse import dve_ops
    from concourse.dve_spec import (
        Spec, Src0, C0, C1, C2, C3, sq, lower, _spill_c3_to_src1,
    )
    from concourse.dve_uop import DveOpSpec
    from concourse.dve_table_gen import dve_ver_for

    name = "EXP_POLY4_ANT"
    for op in dve_ops.OPS:
        if op.name == name:
            _cache["exp_op"] = op
            return op

    def _ref(in0, in1, s0, s1, imm2):
        p = ((s0 * in0 + s1) * in0 + imm2) * in0 + in1
        return (p * p) * (p * p)

    body = sq(sq(((C0 * Src0 + C1) * Src0 + C2) * Src0 + C3))
    spec = Spec(body=_spill_c3_to_src1(body), reference=_ref)
    dve_ops._SUB_OPCODE_FOR_NAME[name] = dve_ops._CUSTOM_DVE_ROW_BASE + len(dve_ops.OPS)
    shas = {}
    for ver in ("v3", "v4"):
        try:
            tmp = DveOpSpec(name=name, opcode=dve_ops.get_dve_sub_opcode(name),
                            uops=lower(spec, ver=ver), rd1_en=True)
            shas[ver] = tmp.sha(ver)
        except Exception:
            pass
    op = dve_ops.DveOp(name, spec, subdim=False, uops_sha=shas)
    dve_ops.OPS.append(op)
    dve_ops.CUSTOM_DVE_SPECS[name] = spec
    _cache["exp_op"] = op
    return op


def _build_program():
    from concourse import bacc, mybir, tile

    exp_op = _register_exp_op()

    f32 = mybir.dt.float32
    bf16 = mybir.dt.bfloat16
    Exp = mybir.ActivationFunctionType.Exp
    Ident = mybir.ActivationFunctionType.Identity

    nc = bacc.Bacc("TRN2", target_bir_lowering=False, debug=False)

    qT = nc.dram_tensor("qT", [D, SQ], bf16, kind="ExternalInput")
    kT = nc.dram_tensor("kT", [D, S], bf16, kind="ExternalInput")
    vT = nc.dram_tensor("vT", [D, S], bf16, kind="ExternalInput")
    wq2 = nc.dram_tensor("wq2", [128, 128], bf16, kind="ExternalInput")
    wk2 = nc.dram_tensor("wk2", [128, 128], bf16, kind="ExternalInput")
    wva = nc.dram_tensor("wva", [128, VAW], bf16, kind="ExternalInput")
    bq2 = nc.dram_tensor("bq2", [128, 1], f32, kind="ExternalInput")
    bk2 = nc.dram_tensor("bk2", [128, 1], f32, kind="ExternalInput")
    bva = nc.dram_tensor("bva", [128, VAW], f32, kind="ExternalInput")
    woT = nc.dram_tensor("woT", [D, D], bf16, kind="ExternalInput")
    bod = nc.dram_tensor("bod", [D, 1], f32, kind="ExternalInput")
    outT = nc.dram_tensor("outT", [D, SQ], f32, kind="ExternalOutput")

    with tile.TileContext(nc) as tc:
        with ExitStack() as ctx:
            ep = ctx.enter_context
            consts = ep(tc.tile_pool(name="consts", bufs=1))
            raw = ep(tc.tile_pool(name="raw", bufs=2))
            projq = ep(tc.tile_pool(name="projq", bufs=2))
            projk = ep(tc.tile_pool(name="projk", bufs=2))
            projv = ep(tc.tile_pool(name="projv", bufs=2))
            attn_p = ep(tc.tile_pool(name="attn", bufs=4))
            norm_p = ep(tc.tile_pool(name="norm", bufs=2))
            hid_p = ep(tc.tile_pool(name="hid", bufs=1))
            outs_p = ep(tc.tile_pool(name="outs", bufs=2))
            sc_ps = ep(tc.tile_pool(name="scps", bufs=2, space="PSUM"))
            av_ps = ep(tc.tile_pool(name="avps", bufs=2, space="PSUM"))

            def mm512(out, lhsT, rhs, start=True, stop=True):
                n = out.shape[-1]
                assert rhs.shape[-1] == n
                for j in range(0, n, 512):
                    w = min(512, n - j)
                    nc.tensor.matmul(out[..., j:j + w], lhsT, rhs[..., j:j + w],
                                     start=start, stop=stop)

            # ---- constants ----
            woT_s = consts.tile([128, 8, D], bf16, tag="woT")
            nc.sync.dma_start(woT_s[:], woT.rearrange("(et p) o -> p et o", p=128))
            bo_s = consts.tile([128, 8], f32, tag="bo")
            nc.sync.dma_start(bo_s[:], bod.rearrange("(ot p) one -> p (ot one)", p=128))
            wq2_s = consts.tile([128, 128], bf16, tag="wq2")
            nc.sync.dma_start(wq2_s[:], wq2[:, :])
            wk2_s = consts.tile([128, 128], bf16, tag="wk2")
            nc.sync.dma_start(wk2_s[:], wk2[:, :])
            wva_s = consts.tile([128, VAW], bf16, tag="wva")
            nc.sync.dma_start(wva_s[:], wva[:, :])
            bq2_s = consts.tile([128, 1], f32, tag="bq2")
            nc.sync.dma_start(bq2_s[:], bq2[:, :])
            bk2_s = consts.tile([128, 1], f32, tag="bk2")
            nc.sync.dma_start(bk2_s[:], bk2[:, :])
            bva_s = consts.tile([128, VAW], f32, tag="bva")
            nc.sync.dma_start(bva_s[:], bva[:, :])
            c3t = consts.tile([128, 1], f32, tag="c3t")
            nc.vector.memset(c3t[:], EXPC0)

            hidden = hid_p.tile([128, 8, SQ], bf16, tag="hidden")

            # ---- PE warm-up: ~5us of back-to-back matmuls flips the HAM
            # clock gate to 8/8 (2.4 GHz) before real work arrives; after
            # that only >3.4us idle windows re-throttle.
            warm = sc_ps.tile([128, 512], f32, tag="sc")
            for _ in range(12):
                nc.tensor.matmul(warm[:], woT_s[:, 0, 0:128],
                                 woT_s[:, 1, 0:512], start=True, stop=True)

            for pair in range(NPAIR):
                rows = slice(pair * 128, (pair + 1) * 128)
                # ---- stream raw inputs (transposed layout) ----
                q2 = raw.tile([128, SQ], bf16, tag="q2")
                nc.sync.dma_start(q2[:], qT[rows, :])
                k2 = raw.tile([128, S], bf16, tag="k2")
                nc.sync.dma_start(k2[:], kT[rows, :])
                v2 = raw.tile([128, S], bf16, tag="v2")
                nc.sync.dma_start(v2[:], vT[rows, :])

                # ---- Q projection: qh^T[e2, q]  (bias-add on ScalarE) ----
                qh = projq.tile([128, SQ], bf16, tag="qh")
                ps = sc_ps.tile([128, SQ], f32, tag="sc")
                mm512(ps[:], wq2_s[:], q2[:])
                nc.scalar.activation(qh[:], ps[:], Ident, bias=bq2_s[:])

                # ---- K projection: kh^T[e2, k] ----
                kh = projk.tile([128, S], bf16, tag="kh")
                for half in range(2):
                    ps = sc_ps.tile([128, SQ], f32, tag="sc")
                    mm512(ps[:], wk2_s[:],
                          k2[:, half * 1024:(half + 1) * 1024])
                    nc.scalar.activation(
                        kh[:, half * 1024:(half + 1) * 1024], ps[:], Ident,
                        bias=bk2_s[:])

                # ---- V projection (natural layout, with ones columns) ----
                vha = projv.tile([128, NKC, VAW], bf16, tag="vha")
                for sc_i in range(NKC):
                    psv = sc_ps.tile([128, VAW], f32, tag="sc")
                    nc.tensor.matmul(
                        psv[:], v2[:, sc_i * 128:(sc_i + 1) * 128], wva_s[:],
                        start=True, stop=True)
                    nc.vector.tensor_tensor(vha[:, sc_i, :], psv[:], bva_s[:],
                                            op=mybir.AluOpType.add)

                # ---- attention over this head pair ----
                avA = av_ps.tile([65, SQ], f32, tag="av")
                avB = av_ps.tile([65, SQ], f32, tag="av")
                for kc in range(NKC):
                    ks = slice(kc * 128, (kc + 1) * 128)
                    scA = sc_ps.tile([128, SQ], f32, tag="sc")
                    scB = sc_ps.tile([128, SQ], f32, tag="sc")
                    mm512(scA[:], kh[0:64, ks], qh[0:64, :])
                    mm512(scB[:], kh[64:128, ks], qh[64:128, :])
                    atA = attn_p.tile([128, SQ], bf16, tag="attn")
                    nc.scalar.activation(atA[:], scA[:], Exp, scale=0.125)
                    atB = attn_p.tile([128, SQ], bf16, tag="attn")
                    if kc < 3 or kc % 3 == 2:
                        # ACT takes both exps early in the pair (DVE drains
                        # its vha/norm backlog) and ~1/3 of later chunks
                        nc.scalar.activation(atB[:], scB[:], Exp, scale=0.125)
                    else:
                        nc.vector._custom_dve(
                            exp_op, out=atB[:], in0=scB[:], in1=c3t[:],
                            s0=EXPC3, s1=EXPC2, imm2=EXPC1)
                    first, last = kc == 0, kc == NKC - 1
                    mm512(avA[:], vha[:, kc, 0:65], atA[:],
                          start=first, stop=last)
                    mm512(avB[:], vha[:, kc, 65:130], atB[:],
                          start=first, stop=last)

                # ---- normalize: hidden^T[e, q] = av[e, q] * (1/av[64, q]) ----
                # Engines are partition-aligned, so the sum row (partition 64)
                # moves to partition 0 via an aligned ScalarE copy + DMA; head
                # B's product is staged at partitions 0:64 and DMA'd into
                # hidden partitions 64:128.
                for half, av in ((0, avA), (1, avB)):
                    rb = norm_p.tile([65, SQ], f32, tag="rb")
                    nc.scalar.copy(rb[64:65, :], av[64:65, :])
                    sums = norm_p.tile([1, SQ], f32, tag="sums")
                    nc.sync.dma_start(sums[:], rb[64:65, :])
                    recip = norm_p.tile([1, SQ], f32, tag="recip")
                    nc.vector.reciprocal_approx_fast(recip[:], sums[:])
                    nc.gpsimd.partition_broadcast(rb[0:64, :], recip[:])
                    if half == 0:
                        nc.vector.tensor_tensor(
                            hidden[0:64, pair, :],
                            av[0:64, :], rb[0:64, :], op=mybir.AluOpType.mult)
                    else:
                        stg = norm_p.tile([64, SQ], bf16, tag="stg")
                        nc.vector.tensor_tensor(
                            stg[:], av[0:64, :], rb[0:64, :],
                            op=mybir.AluOpType.mult)
                        nc.sync.dma_start(hidden[64:128, pair, :], stg[:])

            # ---- output projection: out^T[o, q] ----
            for ot in range(8):
                pso = sc_ps.tile([128, SQ], f32, tag="sc")
                for et in range(8):
                    mm512(pso[:],
                          woT_s[:, et, ot * 128:(ot + 1) * 128],
                          hidden[:, et, :],
                          start=(et == 0), stop=(et == 7))
                o_s = outs_p.tile([128, SQ], f32, tag="outs")
                nc.scalar.activation(o_s[:], pso[:], Ident,
                                     bias=bo_s[:, ot:ot + 1])
                nc.sync.dma_start(outT[ot * 128:(ot + 1) * 128, :], o_s[:])

    nc.compile()
    return nc


def _get_nc():
    if "nc" not in _cache:
        _cache["nc"] = _build_program()
    return _cache["nc"]


def _prep_consts(Wq, bq, Wk, bk, Wv, bv, Wo, bo):
    f = np.float32

    def blockdiag2(W):
        out = np.zeros((128, 128), f)
        out[:64, :64] = W.T
        out[64:, 64:] = W.T
        return out

    wva = np.zeros((128, VAW), f)
    wva[:64, 0:64] = Wv.T          # head A
    wva[64:, 65:129] = Wv.T        # head B
    bva_row = np.zeros((VAW,), f)
    bva_row[0:64] = bv
    bva_row[64] = 1.0
    bva_row[65:129] = bv
    bva_row[129] = 1.0
    import ml_dtypes
    b16 = ml_dtypes.bfloat16
    return {
        "wq2": blockdiag2(Wq).astype(b16),
        "wk2": blockdiag2(Wk).astype(b16),
        "wva": wva.astype(b16),
        "bq2": np.tile(bq.astype(f), 2)[:, None].copy(),
        "bk2": np.tile(bk.astype(f), 2)[:, None].copy(),
        "bva": np.broadcast_to(bva_row, (128, VAW)).copy(),
        "woT": np.ascontiguousarray(Wo.T.astype(f)).astype(b16),
        "bod": bo.astype(f)[:, None].copy(),
    }


def kernel(q, k, v, Wq, bq, Wk, bk, Wv, bv, Wo, bo, _trace=False):
    import ml_dtypes
    b16 = ml_dtypes.bfloat16
    q = np.asarray(q, np.float32)
    k = np.asarray(k, np.float32)
    v = np.asarray(v, np.float32)
    consts = _prep_consts(
        np.asarray(Wq, np.float32), np.asarray(bq, np.float32),
        np.asarray(Wk, np.float32), np.asarray(bk, np.float32),
        np.asarray(Wv, np.float32), np.asarray(bv, np.float32),
        np.asarray(Wo, np.float32), np.asarray(bo, np.float32))

    in_maps = []
    for c in range(N_CORES):
        b, chunk = c // 2, c % 2
        m = dict(consts)
        m["qT"] = np.ascontiguousarray(
            q[b, chunk * SQ:(chunk + 1) * SQ, :].T).astype(b16)
        m["kT"] = np.ascontiguousarray(k[b].T).astype(b16)
        m["vT"] = np.ascontiguousarray(v[b].T).astype(b16)
        in_maps.append(m)

    nc = _get_nc()
    from concourse.bass_utils import run_bass_kernel_spmd
    res = run_bass_kernel_spmd(nc, in_maps, core_ids=list(range(N_CORES)),
                               trace=_trace)
    if _trace:
        kernel.last_results = res

    out = np.empty((B, S, D), np.float32)
    for c in range(N_CORES):
        b, chunk = c // 2, c % 2
        out[b, chunk * SQ:(chunk + 1) * SQ, :] = res.results[c]["outT"].T
    return out



# revision 5
# speedup vs baseline: 1.0517x; 1.0517x over previous
"""Multi-head attention Trainium2 kernel (8 NeuronCores, SPMD).

Problem: B=4, S=2048, D_MODEL=1024, H=16, DIM=64 (nn_MultiHeadAttn).
Sharding: core c handles (batch b = c//2, query-row chunk c%2 of 1024).
Each core computes all 16 heads for its 1024 query rows against the full
2048 keys of its batch, then its rows of the output projection.

v2 — restructured to keep the PE HAM clock gate at 8/8 (2.4 GHz):
  - Scores for the head pair are emitted interleaved (A0,B0,A1,B1).  Head A
    occupies PE row-groups 0-1 (contraction partitions 0:64) and head B
    row-groups 2-3, so consecutive instructions execute CONCURRENTLY in the
    128x128 array (per-subarray row tiling) -- ~2x scores throughput.
  - attn@V runs 2 chunks behind the scores stream so the exp engines
    (ScalarE spline exp + VectorE custom poly exp) always have slack and
    the PE never micro-idles (micro-idles re-throttle HAM to 4/8).
  - The per-pair [65,SQ] PSUM accumulators (64 v-dims + sum-of-exp row) are
    copied whole to SBUF by ScalarE right after the last accumulation,
    freeing PSUM banks in ~1us; softmax normalization (fast reciprocal,
    partition-broadcast, multiply) then runs on VectorE+GpSimdE from SBUF.
  - V bias is folded into the output-projection bias host-side
    (out = (attn@(Wv x))/Z @ Wo^T + [bo + Wo @ tile(bv,H)]), removing a
    per-chunk bias pass.
  - Output projection accumulates e-tiles 0..6 first and e-tile 7 (the last
    pair) last, so it starts while the final pair is still normalizing.
"""

import sys

if "/opt/trn_rl_repo" not in sys.path:
    sys.path.insert(0, "/opt/trn_rl_repo")

import numpy as np
from contextlib import ExitStack

N_CORES = 8
B, S, D = 4, 2048, 1024
H, DIM = 16, 64
SQ = 1024          # query rows per core
NPAIR = 8          # head pairs
NKC = S // 128     # key chunks of 128
VAW = 130          # vha width: (64 v-dims + ones) * 2 heads
AV_LAG = 2         # attn@V trails the scores stream by this many chunks

# deg-3 minimax fit of exp(x/32) on |x|<=20; kernel computes p(x)^4=exp(x/8).
EXPC3 = 4.98779571e-06
EXPC2 = 5.03750782e-04
EXPC1 = 3.13034249e-02
EXPC0 = 9.99313241e-01

# chunks whose head-B exp runs on ScalarE (the rest go to VectorE's custom
# poly op); head A always runs on ScalarE.  Balance: ACT 18, DVE 14 tiles.
ACT_B_CHUNKS = (0, 1)

_cache = {}


def _register_exp_op():
    """Register the custom DVE exp op (deg-3 Horner + 2 squarings, 8 ALU
    stages) in concourse's custom-DVE registry; the per-NEFF uop table is
    generated from dve_ops.OPS at compile time."""
    if "exp_op" in _cache:
        return _cache["exp_op"]
    from concourse import dve_ops
    from concourse.dve_spec import (
        Spec, Src0, C0, C1, C2, C3, sq, lower, _spill_c3_to_src1,
    )
    from concourse.dve_uop import DveOpSpec
    from concourse.dve_table_gen import dve_ver_for

    name = "EXP_POLY4_ANT"
    for op in dve_ops.OPS:
        if op.name == name:
            _cache["exp_op"] = op
            return op

    def _ref(in0, in1, s0, s1, imm2):
        p = ((s0 * in0 + s1) * in0 + imm2) * in0 + in1
        return (p * p) * (p * p)

    body = sq(sq(((C0 * Src0 + C1) * Src0 + C2) * Src0 + C3))
    spec = Spec(body=_spill_c3_to_src1(body), reference=_ref)
    dve_ops._SUB_OPCODE_FOR_NAME[name] = dve_ops._CUSTOM_DVE_ROW_BASE + len(dve_ops.OPS)
    shas = {}
    for ver in ("v3", "v4"):
        try:
            tmp = DveOpSpec(name=name, opcode=dve_ops.get_dve_sub_opcode(name),
                            uops=lower(spec, ver=ver), rd1_en=True)
            shas[ver] = tmp.sha(ver)
        except Exception:
            pass
    op = dve_ops.DveOp(name, spec, subdim=False, uops_sha=shas)
    dve_ops.OPS.append(op)
    dve_ops.CUSTOM_DVE_SPECS[name] = spec
    _cache["exp_op"] = op
    return op


def _build_program():
    from concourse import bacc, mybir, tile

    exp_op = _register_exp_op()

    f32 = mybir.dt.float32
    bf16 = mybir.dt.bfloat16
    Exp = mybir.ActivationFunctionType.Exp
    Ident = mybir.ActivationFunctionType.Identity

    nc = bacc.Bacc("TRN2", target_bir_lowering=False, debug=False)

    qT = nc.dram_tensor("qT", [D, SQ], bf16, kind="ExternalInput")
    kT = nc.dram_tensor("kT", [D, S], bf16, kind="ExternalInput")
    vT = nc.dram_tensor("vT", [D, S], bf16, kind="ExternalInput")
    wq2 = nc.dram_tensor("wq2", [128, 128], bf16, kind="ExternalInput")
    wk2 = nc.dram_tensor("wk2", [128, 128], bf16, kind="ExternalInput")
    wv2 = nc.dram_tensor("wv2", [128, 128], bf16, kind="ExternalInput")
    bq2 = nc.dram_tensor("bq2", [128, 1], f32, kind="ExternalInput")
    bk2 = nc.dram_tensor("bk2", [128, 1], f32, kind="ExternalInput")
    woT = nc.dram_tensor("woT", [D, D], bf16, kind="ExternalInput")
    bod = nc.dram_tensor("bod", [D, 1], f32, kind="ExternalInput")
    outT = nc.dram_tensor("outT", [D, SQ], f32, kind="ExternalOutput")

    with tile.TileContext(nc) as tc:
        with ExitStack() as ctx:
            ep = ctx.enter_context
            consts = ep(tc.tile_pool(name="consts", bufs=1))
            raw = ep(tc.tile_pool(name="raw", bufs=2))
            projq = ep(tc.tile_pool(name="projq", bufs=2))
            projk = ep(tc.tile_pool(name="projk", bufs=2))
            projv = ep(tc.tile_pool(name="projv", bufs=2))
            attn_p = ep(tc.tile_pool(name="attn", bufs=2 * (AV_LAG + 1)))
            norm_p = ep(tc.tile_pool(name="norm", bufs=2))
            hid_p = ep(tc.tile_pool(name="hid", bufs=1))
            outs_p = ep(tc.tile_pool(name="outs", bufs=2))
            sc_ps = ep(tc.tile_pool(name="scps", bufs=2, space="PSUM"))
            av_ps = ep(tc.tile_pool(name="avps", bufs=2, space="PSUM"))

            def mm512(out, lhsT, rhs, start=True, stop=True):
                n = out.shape[-1]
                assert rhs.shape[-1] == n
                for j in range(0, n, 512):
                    w = min(512, n - j)
                    nc.tensor.matmul(out[..., j:j + w], lhsT, rhs[..., j:j + w],
                                     start=start, stop=stop)

            # ---- small constants first (so warm-up + projections can start
            # while the big woT DMA streams in) ----
            wq2_s = consts.tile([128, 128], bf16, tag="wq2")
            nc.sync.dma_start(wq2_s[:], wq2[:, :])
            wk2_s = consts.tile([128, 128], bf16, tag="wk2")
            nc.sync.dma_start(wk2_s[:], wk2[:, :])
            wv2_s = consts.tile([128, 128], bf16, tag="wv2")
            nc.sync.dma_start(wv2_s[:], wv2[:, :])
            bq2_s = consts.tile([128, 1], f32, tag="bq2")
            nc.sync.dma_start(bq2_s[:], bq2[:, :])
            bk2_s = consts.tile([128, 1], f32, tag="bk2")
            nc.sync.dma_start(bk2_s[:], bk2[:, :])
            c3t = consts.tile([128, 1], f32, tag="c3t")
            nc.vector.memset(c3t[:], EXPC0)

            hidden = hid_p.tile([128, 8, SQ], bf16, tag="hidden")

            # ---- PE warm-up: >3.4us of back-to-back matmuls flips the HAM
            # clock gate to 8/8 (2.4 GHz) before real work arrives.  Runs on
            # the small weight tiles while the pair-0 DMAs stream.
            warm = sc_ps.tile([128, SQ], f32, tag="sc")
            for i in range(36):
                nc.tensor.matmul(warm[:, 0:128], wq2_s[:], wk2_s[:],
                                 start=(i == 0), stop=(i == 35))

            woT_s = consts.tile([128, 8, D], bf16, tag="woT")
            nc.sync.dma_start(woT_s[:], woT.rearrange("(et p) o -> p et o", p=128))
            bo_s = consts.tile([128, 8], f32, tag="bo")
            nc.sync.dma_start(bo_s[:], bod.rearrange("(ot p) one -> p (ot one)", p=128))

            for pair in range(NPAIR):
                rows = slice(pair * 128, (pair + 1) * 128)
                # ---- stream raw inputs (transposed layout) ----
                q2 = raw.tile([128, SQ], bf16, tag="q2")
                nc.sync.dma_start(q2[:], qT[rows, :])
                k2 = raw.tile([128, S], bf16, tag="k2")
                nc.sync.dma_start(k2[:], kT[rows, :])
                v2 = raw.tile([128, S], bf16, tag="v2")
                nc.sync.dma_start(v2[:], vT[rows, :])

                # ---- Q projection: qh^T[e2, q]  (bias-add on ScalarE) ----
                qh = projq.tile([128, SQ], bf16, tag="qh")
                ps = sc_ps.tile([128, SQ], f32, tag="sc")
                mm512(ps[:], wq2_s[:], q2[:])
                nc.scalar.activation(qh[:], ps[:], Ident, bias=bq2_s[:])

                # ---- K projection: kh^T[e2, k] ----
                kh = projk.tile([128, S], bf16, tag="kh")
                for half in range(2):
                    ps = sc_ps.tile([128, SQ], f32, tag="sc")
                    mm512(ps[:], wk2_s[:],
                          k2[:, half * 1024:(half + 1) * 1024])
                    nc.scalar.activation(
                        kh[:, half * 1024:(half + 1) * 1024], ps[:], Ident,
                        bias=bk2_s[:])

                # ---- V projection (no bias -- folded into out-proj bias).
                # 4 chunks share one PSUM bank-group; one strided VectorE
                # copy moves them into the vha layout; ones columns memset.
                vha = projv.tile([128, NKC, VAW], bf16, tag="vha")
                nc.vector.memset(vha[:, :, 64:65], 1.0)
                nc.vector.memset(vha[:, :, 129:130], 1.0)
                for g in range(NKC // 4):
                    psv = sc_ps.tile([128, 512], f32, tag="sc")
                    for c in range(4):
                        sc_i = g * 4 + c
                        nc.tensor.matmul(
                            psv[:, c * 128:(c + 1) * 128],
                            v2[:, sc_i * 128:(sc_i + 1) * 128], wv2_s[:],
                            start=True, stop=True)
                    src = psv[:].rearrange("p (c h e) -> p c h e", c=4, e=64)
                    dst = vha[:, g * 4:(g + 1) * 4, :].rearrange(
                        "p c (h e) -> p c h e", e=65)[:, :, :, 0:64]
                    nc.vector.tensor_copy(dst, src)

                # ---- attention over this head pair ----
                # Scores interleave head A (PE rows 0:64) and head B (rows
                # 64:128) so the row-tiled matmuls run concurrently; attn@V
                # trails AV_LAG chunks so exp never stalls the PE.
                avA = av_ps.tile([65, SQ], f32, tag="av")
                avB = av_ps.tile([65, SQ], f32, tag="av")
                ats = {}

                def do_av(kc):
                    first, last = kc == 0, kc == NKC - 1
                    atA, atB = ats.pop(kc)
                    mm512(avA[:], vha[:, kc, 0:65], atA[:],
                          start=first, stop=last)
                    mm512(avB[:], vha[:, kc, 65:130], atB[:],
                          start=first, stop=last)

                for kc in range(NKC):
                    ks = slice(kc * 128, (kc + 1) * 128)
                    scA = sc_ps.tile([128, SQ], f32, tag="sc")
                    scB = sc_ps.tile([128, SQ], f32, tag="sc")
                    for j in (0, 512):
                        nc.tensor.matmul(scA[:, j:j + 512], kh[0:64, ks],
                                         qh[0:64, j:j + 512],
                                         start=True, stop=True)
                        nc.tensor.matmul(scB[:, j:j + 512], kh[64:128, ks],
                                         qh[64:128, j:j + 512],
                                         start=True, stop=True)
                    # exp in 512-col halves: subtile deps let the next
                    # chunk's scores overwrite each half as soon as it has
                    # been read, so the PE never waits on a full-tile read.
                    atA = attn_p.tile([128, SQ], bf16, tag="attn")
                    for j in (0, 512):
                        nc.scalar.activation(atA[:, j:j + 512],
                                             scA[:, j:j + 512], Exp,
                                             scale=0.125)
                    atB = attn_p.tile([128, SQ], bf16, tag="attn")
                    for j in (0, 512):
                        if kc in ACT_B_CHUNKS:
                            nc.scalar.activation(atB[:, j:j + 512],
                                                 scB[:, j:j + 512], Exp,
                                                 scale=0.125)
                        else:
                            nc.vector._custom_dve(
                                exp_op, out=atB[:, j:j + 512],
                                in0=scB[:, j:j + 512], in1=c3t[:],
                                s0=EXPC3, s1=EXPC2, imm2=EXPC1)
                    ats[kc] = (atA, atB)
                    if kc >= AV_LAG:
                        do_av(kc - AV_LAG)
                for kc in range(NKC - AV_LAG, NKC):
                    do_av(kc)

                # ---- normalize: hidden^T[e, q] = av[e, q] / av[64, q] ----
                # ScalarE copies the whole accumulator to SBUF (frees the
                # PSUM banks ~1us after the last matmul); the reciprocal /
                # broadcast / multiply chain then runs off-PSUM.
                for half, av in ((0, avA), (1, avB)):
                    avs = norm_p.tile([65, SQ], f32, tag="avs")
                    nc.scalar.copy(avs[:], av[:])
                    sums = norm_p.tile([1, SQ], f32, tag="sums")
                    nc.sync.dma_start(sums[:], avs[64:65, :])
                    recip = norm_p.tile([1, SQ], f32, tag="recip")
                    nc.vector.reciprocal_approx_fast(recip[:], sums[:])
                    fac = norm_p.tile([64, SQ], f32, tag="fac")
                    nc.gpsimd.partition_broadcast(fac[:], recip[:])
                    if half == 0:
                        nc.gpsimd.tensor_tensor(
                            hidden[0:64, pair, :], avs[0:64, :], fac[:],
                            op=mybir.AluOpType.mult)
                    else:
                        stg = norm_p.tile([64, SQ], bf16, tag="stg")
                        nc.gpsimd.tensor_tensor(
                            stg[:], avs[0:64, :], fac[:],
                            op=mybir.AluOpType.mult)
                        nc.sync.dma_start(hidden[64:128, pair, :], stg[:])

            # ---- output projection: out^T[o, q] ----
            # e-tile 7 (the final pair) accumulates LAST so the projection
            # overlaps the final pair's normalize chain.
            et_order = list(range(7)) + [7]
            for ot in range(8):
                pso = sc_ps.tile([128, SQ], f32, tag="sc")
                for i, et in enumerate(et_order):
                    mm512(pso[:],
                          woT_s[:, et, ot * 128:(ot + 1) * 128],
                          hidden[:, et, :],
                          start=(i == 0), stop=(i == 7))
                o_s = outs_p.tile([128, SQ], f32, tag="outs")
                nc.scalar.activation(o_s[:], pso[:], Ident,
                                     bias=bo_s[:, ot:ot + 1])
                nc.sync.dma_start(outT[ot * 128:(ot + 1) * 128, :], o_s[:])

    nc.compile()
    return nc


def _get_nc():
    if "nc" not in _cache:
        _cache["nc"] = _build_program()
    return _cache["nc"]


def _prep_consts(Wq, bq, Wk, bk, Wv, bv, Wo, bo):
    f = np.float32

    def blockdiag2(W):
        out = np.zeros((128, 128), f)
        out[:64, :64] = W.T
        out[64:, 64:] = W.T
        return out

    import ml_dtypes
    b16 = ml_dtypes.bfloat16
    # V bias folded into the output-projection bias: hidden rows carry
    # (attn @ Wv x)/Z only, and out = hidden @ Wo^T + (bo + Wo @ tile(bv, H)).
    bo_fold = bo.astype(f) + Wo.astype(f) @ np.tile(bv.astype(f), H)
    return {
        "wq2": blockdiag2(Wq).astype(b16),
        "wk2": blockdiag2(Wk).astype(b16),
        "wv2": blockdiag2(Wv).astype(b16),
        "bq2": np.tile(bq.astype(f), 2)[:, None].copy(),
        "bk2": np.tile(bk.astype(f), 2)[:, None].copy(),
        "woT": np.ascontiguousarray(Wo.T.astype(f)).astype(b16),
        "bod": bo_fold[:, None].copy(),
    }


def kernel(q, k, v, Wq, bq, Wk, bk, Wv, bv, Wo, bo, _trace=False):
    import ml_dtypes
    b16 = ml_dtypes.bfloat16
    q = np.asarray(q, np.float32)
    k = np.asarray(k, np.float32)
    v = np.asarray(v, np.float32)
    consts = _prep_consts(
        np.asarray(Wq, np.float32), np.asarray(bq, np.float32),
        np.asarray(Wk, np.float32), np.asarray(bk, np.float32),
        np.asarray(Wv, np.float32), np.asarray(bv, np.float32),
        np.asarray(Wo, np.float32), np.asarray(bo, np.float32))

    in_maps = []
    for c in range(N_CORES):
        b, chunk = c // 2, c % 2
        m = dict(consts)
        m["qT"] = np.ascontiguousarray(
            q[b, chunk * SQ:(chunk + 1) * SQ, :].T).astype(b16)
        m["kT"] = np.ascontiguousarray(k[b].T).astype(b16)
        m["vT"] = np.ascontiguousarray(v[b].T).astype(b16)
        in_maps.append(m)

    nc = _get_nc()
    from concourse.bass_utils import run_bass_kernel_spmd
    res = run_bass_kernel_spmd(nc, in_maps, core_ids=list(range(N_CORES)),
                               trace=_trace)
    if _trace:
        kernel.last_results = res

    out = np.empty((B, S, D), np.float32)
    for c in range(N_CORES):
        b, chunk = c // 2, c % 2
        out[b, chunk * SQ:(chunk + 1) * SQ, :] = res.results[c]["outT"].T
    return out


# revision 12
# speedup vs baseline: 1.4465x; 1.3754x over previous
"""Multi-head attention Trainium2 kernel (8 NeuronCores, SPMD).

Problem: B=4, S=2048, D_MODEL=1024, H=16, DIM=64 (nn_MultiHeadAttn).
Sharding: core c handles (batch b = c//2, query-row chunk c%2 of 1024).
Each core computes all 16 heads for its 1024 query rows against the full
2048 keys of its batch, then its rows of the output projection.

v3 — restructured to keep the PE HAM clock gate at 8/8 (2.4 GHz):
  - EVERY matmul runs in 128-row tile mode.  The Q projection produces
    zero-padded per-head tiles (qhA rows 64:128 are exact zeros from a zero
    weight block, and vice versa), so scores contract the full 128-partition
    kh tile.  Mixing 64-row scores with 128-row attn@V (v2) forced a PE
    tile-mode switch every chunk; each switch drains the systolic array and
    the resulting micro-bubbles keep the HAM activity monitor demoting the
    clock to 4/8 (measured: 86% of the run at 1.2 GHz).
  - attn@V runs 2 chunks behind the scores stream so the exp engines
    (ScalarE spline exp + VectorE custom poly exp) always have slack and
    the PE never micro-idles (micro-idles re-throttle HAM to 4/8).
  - The per-pair [65,SQ] PSUM accumulators (64 v-dims + sum-of-exp row) are
    copied whole to SBUF by ScalarE right after the last accumulation,
    freeing PSUM banks in ~1us; softmax normalization (fast reciprocal,
    partition-broadcast, multiply) then runs on VectorE+GpSimdE from SBUF.
  - V bias is folded into the output-projection bias host-side
    (out = (attn@(Wv x))/Z @ Wo^T + [bo + Wo @ tile(bv,H)]), removing a
    per-chunk bias pass.
  - Output projection accumulates e-tiles 0..6 first and e-tile 7 (the last
    pair) last, so it starts while the final pair is still normalizing.
"""

import sys

if "/opt/trn_rl_repo" not in sys.path:
    sys.path.insert(0, "/opt/trn_rl_repo")

import numpy as np
from contextlib import ExitStack

N_CORES = 8
B, S, D = 4, 2048, 1024
H, DIM = 16, 64
SQ = 1024          # query rows per core
NPAIR = 8          # head pairs
NKC = S // 128     # key chunks of 128
VAW = 130          # vha width: (64 v-dims + ones) * 2 heads
AV_LAG = 2         # attn@V trails the scores stream by this many chunks

# deg-3 minimax fit of exp(x/32) on |x|<=20; kernel computes p(x)^4=exp(x/8).
EXPC3 = 4.98779571e-06
EXPC2 = 5.03750782e-04
EXPC1 = 3.13034249e-02
EXPC0 = 9.99313241e-01

# chunks whose head-B exp runs on ScalarE (the rest go to VectorE's custom
# poly op); head A always runs on ScalarE.  Balance: ACT 18, DVE 14 tiles.
ACT_B_CHUNKS = (0, 1)

_cache = {}


def _register_exp_op():
    """Register the custom DVE exp op (deg-3 Horner + 2 squarings, 8 ALU
    stages) in concourse's custom-DVE registry; the per-NEFF uop table is
    generated from dve_ops.OPS at compile time."""
    if "exp_op" in _cache:
        return _cache["exp_op"]
    from concourse import dve_ops
    from concourse.dve_spec import (
        Spec, Src0, C0, C1, C2, C3, sq, lower, _spill_c3_to_src1,
    )
    from concourse.dve_uop import DveOpSpec
    from concourse.dve_table_gen import dve_ver_for

    name = "EXP_POLY4_ANT"
    for op in dve_ops.OPS:
        if op.name == name:
            _cache["exp_op"] = op
            return op

    def _ref(in0, in1, s0, s1, imm2):
        p = ((s0 * in0 + s1) * in0 + imm2) * in0 + in1
        return (p * p) * (p * p)

    body = sq(sq(((C0 * Src0 + C1) * Src0 + C2) * Src0 + C3))
    spec = Spec(body=_spill_c3_to_src1(body), reference=_ref)
    dve_ops._SUB_OPCODE_FOR_NAME[name] = dve_ops._CUSTOM_DVE_ROW_BASE + len(dve_ops.OPS)
    shas = {}
    for ver in ("v3", "v4"):
        try:
            tmp = DveOpSpec(name=name, opcode=dve_ops.get_dve_sub_opcode(name),
                            uops=lower(spec, ver=ver), rd1_en=True)
            shas[ver] = tmp.sha(ver)
        except Exception:
            pass
    op = dve_ops.DveOp(name, spec, subdim=False, uops_sha=shas)
    dve_ops.OPS.append(op)
    dve_ops.CUSTOM_DVE_SPECS[name] = spec
    _cache["exp_op"] = op
    return op


def _build_program():
    from concourse import bacc, mybir, tile

    exp_op = _register_exp_op()

    f32 = mybir.dt.float32
    bf16 = mybir.dt.bfloat16
    Exp = mybir.ActivationFunctionType.Exp
    Ident = mybir.ActivationFunctionType.Identity

    nc = bacc.Bacc("TRN2", target_bir_lowering=False, debug=False)

    qT = nc.dram_tensor("qT", [D, SQ], bf16, kind="ExternalInput")
    kT = nc.dram_tensor("kT", [D, S], bf16, kind="ExternalInput")
    vT = nc.dram_tensor("vT", [D, S], bf16, kind="ExternalInput")
    wqA = nc.dram_tensor("wqA", [128, 128], bf16, kind="ExternalInput")
    wqB = nc.dram_tensor("wqB", [128, 128], bf16, kind="ExternalInput")
    wk2 = nc.dram_tensor("wk2", [128, 128], bf16, kind="ExternalInput")
    wv2 = nc.dram_tensor("wv2", [128, 128], bf16, kind="ExternalInput")
    bqA2 = nc.dram_tensor("bqA2", [128, 1], f32, kind="ExternalInput")
    bqB2 = nc.dram_tensor("bqB2", [128, 1], f32, kind="ExternalInput")
    bk2 = nc.dram_tensor("bk2", [128, 1], f32, kind="ExternalInput")
    woT = nc.dram_tensor("woT", [D, D], bf16, kind="ExternalInput")
    bod = nc.dram_tensor("bod", [D, 1], f32, kind="ExternalInput")
    outT = nc.dram_tensor("outT", [D, SQ], f32, kind="ExternalOutput")

    with tile.TileContext(nc) as tc:
        with ExitStack() as ctx:
            ep = ctx.enter_context
            consts = ep(tc.tile_pool(name="consts", bufs=1))
            raw = ep(tc.tile_pool(name="raw", bufs=2))
            projq = ep(tc.tile_pool(name="projq", bufs=2))
            projk = ep(tc.tile_pool(name="projk", bufs=2))
            projv = ep(tc.tile_pool(name="projv", bufs=2))
            attn_p = ep(tc.tile_pool(name="attn", bufs=2 * (AV_LAG + 1)))
            norm_p = ep(tc.tile_pool(name="norm", bufs=2))
            hid_p = ep(tc.tile_pool(name="hid", bufs=1))
            outs_p = ep(tc.tile_pool(name="outs", bufs=2))
            sc_ps = ep(tc.tile_pool(name="scps", bufs=2, space="PSUM"))
            av_ps = ep(tc.tile_pool(name="avps", bufs=2, space="PSUM"))

            def mm512(out, lhsT, rhs, start=True, stop=True):
                n = out.shape[-1]
                assert rhs.shape[-1] == n
                for j in range(0, n, 512):
                    w = min(512, n - j)
                    nc.tensor.matmul(out[..., j:j + w], lhsT, rhs[..., j:j + w],
                                     start=start, stop=stop)

            # ---- small constants first (so warm-up + projections can start
            # while the big woT DMA streams in) ----
            wqA_s = consts.tile([128, 128], bf16, tag="wqA")
            nc.sync.dma_start(wqA_s[:], wqA[:, :])
            wqB_s = consts.tile([128, 128], bf16, tag="wqB")
            nc.sync.dma_start(wqB_s[:], wqB[:, :])
            wk2_s = consts.tile([128, 128], bf16, tag="wk2")
            nc.sync.dma_start(wk2_s[:], wk2[:, :])
            wv2_s = consts.tile([128, 128], bf16, tag="wv2")
            nc.sync.dma_start(wv2_s[:], wv2[:, :])
            bqA_s = consts.tile([128, 1], f32, tag="bqA")
            nc.sync.dma_start(bqA_s[:], bqA2[:, :])
            bqB_s = consts.tile([128, 1], f32, tag="bqB")
            nc.sync.dma_start(bqB_s[:], bqB2[:, :])
            bk2_s = consts.tile([128, 1], f32, tag="bk2")
            nc.sync.dma_start(bk2_s[:], bk2[:, :])
            c3t = consts.tile([128, 1], f32, tag="c3t")
            nc.vector.memset(c3t[:], EXPC0)

            hidden = hid_p.tile([128, 8, SQ], bf16, tag="hidden")

            # ---- PE warm-up: >3.4us of back-to-back matmuls flips the HAM
            # clock gate to 8/8 (2.4 GHz) before real work arrives.  Runs on
            # the small weight tiles while the pair-0 DMAs stream.
            warm = sc_ps.tile([128, SQ], f32, tag="sc")
            for i in range(36):
                nc.tensor.matmul(warm[:, 0:128], wqA_s[:], wk2_s[:],
                                 start=(i == 0), stop=(i == 35))

            woT_s = consts.tile([128, 8, D], bf16, tag="woT")
            nc.sync.dma_start(woT_s[:], woT.rearrange("(et p) o -> p et o", p=128))
            bo_s = consts.tile([128, 8], f32, tag="bo")
            nc.sync.dma_start(bo_s[:], bod.rearrange("(ot p) one -> p (ot one)", p=128))

            for pair in range(NPAIR):
                rows = slice(pair * 128, (pair + 1) * 128)
                # ---- stream raw inputs (transposed layout) ----
                q2 = raw.tile([128, SQ], bf16, tag="q2")
                nc.sync.dma_start(q2[:], qT[rows, :])
                k2 = raw.tile([128, S], bf16, tag="k2")
                nc.sync.dma_start(k2[:], kT[rows, :])
                v2 = raw.tile([128, S], bf16, tag="v2")
                nc.sync.dma_start(v2[:], vT[rows, :])

                # ---- Q projection, zero-padded per head ----
                # qhA rows 0:64 = head A projection, rows 64:128 = exact
                # zeros (from the zero weight block); qhB vice versa.  Scores
                # then contract the FULL 128 partitions against kh, so every
                # matmul in the kernel runs in 128-row tile mode -- no PE
                # tile-mode switches (each switch drains the array and the
                # resulting micro-bubbles pin the HAM clock gate at 4/8).
                qhA = projq.tile([128, SQ], bf16, tag="qhA")
                ps = sc_ps.tile([128, SQ], f32, tag="sc")
                mm512(ps[:], wqA_s[:], q2[:])
                nc.scalar.activation(qhA[:], ps[:], Ident, bias=bqA_s[:])
                qhB = projq.tile([128, SQ], bf16, tag="qhB")
                ps = sc_ps.tile([128, SQ], f32, tag="sc")
                mm512(ps[:], wqB_s[:], q2[:])
                nc.scalar.activation(qhB[:], ps[:], Ident, bias=bqB_s[:])

                # ---- K projection: kh^T[e2, k] ----
                kh = projk.tile([128, S], bf16, tag="kh")
                for half in range(2):
                    ps = sc_ps.tile([128, SQ], f32, tag="sc")
                    mm512(ps[:], wk2_s[:],
                          k2[:, half * 1024:(half + 1) * 1024])
                    nc.scalar.activation(
                        kh[:, half * 1024:(half + 1) * 1024], ps[:], Ident,
                        bias=bk2_s[:])

                # ---- V projection (no bias -- folded into out-proj bias).
                # 4 chunks share one PSUM bank-group; one strided VectorE
                # copy moves them into the vha layout; ones columns memset.
                vha = projv.tile([128, NKC, VAW], bf16, tag="vha")
                nc.vector.memset(vha[:, :, 64:65], 1.0)
                nc.vector.memset(vha[:, :, 129:130], 1.0)
                for g in range(NKC // 4):
                    psv = sc_ps.tile([128, 512], f32, tag="sc")
                    for c in range(4):
                        sc_i = g * 4 + c
                        nc.tensor.matmul(
                            psv[:, c * 128:(c + 1) * 128],
                            v2[:, sc_i * 128:(sc_i + 1) * 128], wv2_s[:],
                            start=True, stop=True)
                    src = psv[:].rearrange("p (c h e) -> p c h e", c=4, e=64)
                    dst = vha[:, g * 4:(g + 1) * 4, :].rearrange(
                        "p c (h e) -> p c h e", e=65)[:, :, :, 0:64]
                    nc.vector.tensor_copy(dst, src)

                # ---- attention over this head pair ----
                # Scores interleave head A (PE rows 0:64) and head B (rows
                # 64:128) so the row-tiled matmuls run concurrently; attn@V
                # trails AV_LAG chunks so exp never stalls the PE.
                avA = av_ps.tile([65, SQ], f32, tag="av")
                avB = av_ps.tile([65, SQ], f32, tag="av")
                ats = {}

                def do_av(kc):
                    first, last = kc == 0, kc == NKC - 1
                    atA, atB = ats.pop(kc)
                    mm512(avA[:], vha[:, kc, 0:65], atA[:],
                          start=first, stop=last)
                    mm512(avB[:], vha[:, kc, 65:130], atB[:],
                          start=first, stop=last)

                for kc in range(NKC):
                    ks = slice(kc * 128, (kc + 1) * 128)
                    scA = sc_ps.tile([128, SQ], f32, tag="sc")
                    scB = sc_ps.tile([128, SQ], f32, tag="sc")
                    for j in (0, 512):
                        nc.tensor.matmul(scA[:, j:j + 512], kh[:, ks],
                                         qhA[:, j:j + 512],
                                         start=True, stop=True)
                        nc.tensor.matmul(scB[:, j:j + 512], kh[:, ks],
                                         qhB[:, j:j + 512],
                                         start=True, stop=True)
                    # exp in 512-col halves: subtile deps let the next
                    # chunk's scores overwrite each half as soon as it has
                    # been read, so the PE never waits on a full-tile read.
                    atA = attn_p.tile([128, SQ], bf16, tag="attn")
                    for j in (0, 512):
                        nc.scalar.activation(atA[:, j:j + 512],
                                             scA[:, j:j + 512], Exp,
                                             scale=0.125)
                    atB = attn_p.tile([128, SQ], bf16, tag="attn")
                    for j in (0, 512):
                        if kc in ACT_B_CHUNKS:
                            nc.scalar.activation(atB[:, j:j + 512],
                                                 scB[:, j:j + 512], Exp,
                                                 scale=0.125)
                        else:
                            nc.vector._custom_dve(
                                exp_op, out=atB[:, j:j + 512],
                                in0=scB[:, j:j + 512], in1=c3t[:],
                                s0=EXPC3, s1=EXPC2, imm2=EXPC1)
                    ats[kc] = (atA, atB)
                    if kc >= AV_LAG:
                        do_av(kc - AV_LAG)
                for kc in range(NKC - AV_LAG, NKC):
                    do_av(kc)

                # ---- normalize: hidden^T[e, q] = av[e, q] / av[64, q] ----
                # ScalarE copies the whole accumulator to SBUF (frees the
                # PSUM banks ~1us after the last matmul); the reciprocal /
                # broadcast / multiply chain then runs off-PSUM.
                for half, av in ((0, avA), (1, avB)):
                    avs = norm_p.tile([65, SQ], f32, tag="avs")
                    nc.scalar.copy(avs[:], av[:])
                    sums = norm_p.tile([1, SQ], f32, tag="sums")
                    nc.sync.dma_start(sums[:], avs[64:65, :])
                    recip = norm_p.tile([1, SQ], f32, tag="recip")
                    nc.vector.reciprocal_approx_fast(recip[:], sums[:])
                    fac = norm_p.tile([64, SQ], f32, tag="fac")
                    nc.gpsimd.partition_broadcast(fac[:], recip[:])
                    if half == 0:
                        nc.gpsimd.tensor_tensor(
                            hidden[0:64, pair, :], avs[0:64, :], fac[:],
                            op=mybir.AluOpType.mult)
                    else:
                        stg = norm_p.tile([64, SQ], bf16, tag="stg")
                        nc.gpsimd.tensor_tensor(
                            stg[:], avs[0:64, :], fac[:],
                            op=mybir.AluOpType.mult)
                        nc.sync.dma_start(hidden[64:128, pair, :], stg[:])

            # ---- output projection: out^T[o, q] ----
            # e-tile 7 (the final pair) accumulates LAST so the projection
            # overlaps the final pair's normalize chain.
            et_order = list(range(7)) + [7]
            for ot in range(8):
                pso = sc_ps.tile([128, SQ], f32, tag="sc")
                for i, et in enumerate(et_order):
                    mm512(pso[:],
                          woT_s[:, et, ot * 128:(ot + 1) * 128],
                          hidden[:, et, :],
                          start=(i == 0), stop=(i == 7))
                o_s = outs_p.tile([128, SQ], f32, tag="outs")
                nc.scalar.activation(o_s[:], pso[:], Ident,
                                     bias=bo_s[:, ot:ot + 1])
                nc.sync.dma_start(outT[ot * 128:(ot + 1) * 128, :], o_s[:])

    nc.compile()
    return nc


def _get_nc():
    if "nc" not in _cache:
        _cache["nc"] = _build_program()
    return _cache["nc"]


def _prep_consts(Wq, bq, Wk, bk, Wv, bv, Wo, bo):
    f = np.float32

    def blockdiag2(W):
        out = np.zeros((128, 128), f)
        out[:64, :64] = W.T
        out[64:, 64:] = W.T
        return out

    import ml_dtypes
    b16 = ml_dtypes.bfloat16
    # V bias folded into the output-projection bias: hidden rows carry
    # (attn @ Wv x)/Z only, and out = hidden @ Wo^T + (bo + Wo @ tile(bv, H)).
    bo_fold = bo.astype(f) + Wo.astype(f) @ np.tile(bv.astype(f), H)
    wqA = np.zeros((128, 128), f)
    wqA[:64, :64] = Wq.T
    wqB = np.zeros((128, 128), f)
    wqB[64:, 64:] = Wq.T
    bqA2 = np.zeros((128, 1), f)
    bqA2[:64, 0] = bq
    bqB2 = np.zeros((128, 1), f)
    bqB2[64:, 0] = bq
    return {
        "wqA": wqA.astype(b16),
        "wqB": wqB.astype(b16),
        "wk2": blockdiag2(Wk).astype(b16),
        "wv2": blockdiag2(Wv).astype(b16),
        "bqA2": bqA2,
        "bqB2": bqB2,
        "bk2": np.tile(bk.astype(f), 2)[:, None].copy(),
        "woT": np.ascontiguousarray(Wo.T.astype(f)).astype(b16),
        "bod": bo_fold[:, None].copy(),
    }


def kernel(q, k, v, Wq, bq, Wk, bk, Wv, bv, Wo, bo, _trace=False):
    import ml_dtypes
    b16 = ml_dtypes.bfloat16
    q = np.asarray(q, np.float32)
    k = np.asarray(k, np.float32)
    v = np.asarray(v, np.float32)
    consts = _prep_consts(
        np.asarray(Wq, np.float32), np.asarray(bq, np.float32),
        np.asarray(Wk, np.float32), np.asarray(bk, np.float32),
        np.asarray(Wv, np.float32), np.asarray(bv, np.float32),
        np.asarray(Wo, np.float32), np.asarray(bo, np.float32))

    in_maps = []
    for c in range(N_CORES):
        b, chunk = c // 2, c % 2
        m = dict(consts)
        m["qT"] = np.ascontiguousarray(
            q[b, chunk * SQ:(chunk + 1) * SQ, :].T).astype(b16)
        m["kT"] = np.ascontiguousarray(k[b].T).astype(b16)
        m["vT"] = np.ascontiguousarray(v[b].T).astype(b16)
        in_maps.append(m)

    nc = _get_nc()
    from concourse.bass_utils import run_bass_kernel_spmd
    res = run_bass_kernel_spmd(nc, in_maps, core_ids=list(range(N_CORES)),
                               trace=_trace)
    if _trace:
        kernel.last_results = res

    out = np.empty((B, S, D), np.float32)
    for c in range(N_CORES):
        b, chunk = c // 2, c % 2
        out[b, chunk * SQ:(chunk + 1) * SQ, :] = res.results[c]["outT"].T
    return out


# revision 14
# speedup vs baseline: 1.4803x; 1.0234x over previous
"""Multi-head attention Trainium2 kernel (8 NeuronCores, SPMD).

Problem: B=4, S=2048, D_MODEL=1024, H=16, DIM=64 (nn_MultiHeadAttn).
Sharding: core c handles (batch b = c//2, query-row chunk c%2 of 1024).
Each core computes all 16 heads for its 1024 query rows against the full
2048 keys of its batch, then its rows of the output projection.

v3 — restructured to keep the PE HAM clock gate at 8/8 (2.4 GHz):
  - EVERY matmul runs in 128-row tile mode.  The Q projection produces
    zero-padded per-head tiles (qhA rows 64:128 are exact zeros from a zero
    weight block, and vice versa), so scores contract the full 128-partition
    kh tile.  Mixing 64-row scores with 128-row attn@V (v2) forced a PE
    tile-mode switch every chunk; each switch drains the systolic array and
    the resulting micro-bubbles keep the HAM activity monitor demoting the
    clock to 4/8 (measured: 86% of the run at 1.2 GHz).
  - attn@V runs 2 chunks behind the scores stream so the exp engines
    (ScalarE spline exp + VectorE custom poly exp) always have slack and
    the PE never micro-idles (micro-idles re-throttle HAM to 4/8).
  - The per-pair [65,SQ] PSUM accumulators (64 v-dims + sum-of-exp row) are
    copied whole to SBUF by ScalarE right after the last accumulation,
    freeing PSUM banks in ~1us; softmax normalization (fast reciprocal,
    partition-broadcast, multiply) then runs on VectorE+GpSimdE from SBUF.
  - V bias is folded into the output-projection bias host-side
    (out = (attn@(Wv x))/Z @ Wo^T + [bo + Wo @ tile(bv,H)]), removing a
    per-chunk bias pass.
  - Output projection accumulates e-tiles 0..6 first and e-tile 7 (the last
    pair) last, so it starts while the final pair is still normalizing.
"""

import sys

if "/opt/trn_rl_repo" not in sys.path:
    sys.path.insert(0, "/opt/trn_rl_repo")

import numpy as np
from contextlib import ExitStack

N_CORES = 8
B, S, D = 4, 2048, 1024
H, DIM = 16, 64
SQ = 1024          # query rows per core
NPAIR = 8          # head pairs
NKC = S // 128     # key chunks of 128
VAW = 130          # vha width: (64 v-dims + ones) * 2 heads
AV_LAG = 2         # attn@V trails the scores stream by this many chunks

# deg-3 minimax fit of exp(x/32) on |x|<=20; kernel computes p(x)^4=exp(x/8).
EXPC3 = 4.98779571e-06
EXPC2 = 5.03750782e-04
EXPC1 = 3.13034249e-02
EXPC0 = 9.99313241e-01

_cache = {}


def _register_exp_op():
    """Register the custom DVE exp op (deg-3 Horner + 2 squarings, 8 ALU
    stages) in concourse's custom-DVE registry; the per-NEFF uop table is
    generated from dve_ops.OPS at compile time."""
    if "exp_op" in _cache:
        return _cache["exp_op"]
    from concourse import dve_ops
    from concourse.dve_spec import (
        Spec, Src0, C0, C1, C2, C3, sq, lower, _spill_c3_to_src1,
    )
    from concourse.dve_uop import DveOpSpec
    from concourse.dve_table_gen import dve_ver_for

    name = "EXP_POLY4_ANT"
    for op in dve_ops.OPS:
        if op.name == name:
            _cache["exp_op"] = op
            return op

    def _ref(in0, in1, s0, s1, imm2):
        p = ((s0 * in0 + s1) * in0 + imm2) * in0 + in1
        return (p * p) * (p * p)

    body = sq(sq(((C0 * Src0 + C1) * Src0 + C2) * Src0 + C3))
    spec = Spec(body=_spill_c3_to_src1(body), reference=_ref)
    dve_ops._SUB_OPCODE_FOR_NAME[name] = dve_ops._CUSTOM_DVE_ROW_BASE + len(dve_ops.OPS)
    shas = {}
    for ver in ("v3", "v4"):
        try:
            tmp = DveOpSpec(name=name, opcode=dve_ops.get_dve_sub_opcode(name),
                            uops=lower(spec, ver=ver), rd1_en=True)
            shas[ver] = tmp.sha(ver)
        except Exception:
            pass
    op = dve_ops.DveOp(name, spec, subdim=False, uops_sha=shas)
    dve_ops.OPS.append(op)
    dve_ops.CUSTOM_DVE_SPECS[name] = spec
    _cache["exp_op"] = op
    return op


def _build_program():
    from concourse import bacc, mybir, tile

    exp_op = _register_exp_op()

    f32 = mybir.dt.float32
    bf16 = mybir.dt.bfloat16
    Exp = mybir.ActivationFunctionType.Exp
    Ident = mybir.ActivationFunctionType.Identity

    nc = bacc.Bacc("TRN2", target_bir_lowering=False, debug=False)

    qT = nc.dram_tensor("qT", [D, SQ], bf16, kind="ExternalInput")
    kT = nc.dram_tensor("kT", [D, S], bf16, kind="ExternalInput")
    vT = nc.dram_tensor("vT", [D, S], bf16, kind="ExternalInput")
    wqA = nc.dram_tensor("wqA", [128, 128], bf16, kind="ExternalInput")
    wqB = nc.dram_tensor("wqB", [128, 128], bf16, kind="ExternalInput")
    wk2 = nc.dram_tensor("wk2", [128, 128], bf16, kind="ExternalInput")
    wv2 = nc.dram_tensor("wv2", [128, 128], bf16, kind="ExternalInput")
    bqA2 = nc.dram_tensor("bqA2", [128, 1], f32, kind="ExternalInput")
    bqB2 = nc.dram_tensor("bqB2", [128, 1], f32, kind="ExternalInput")
    bk2 = nc.dram_tensor("bk2", [128, 1], f32, kind="ExternalInput")
    woT = nc.dram_tensor("woT", [D, D], bf16, kind="ExternalInput")
    bod = nc.dram_tensor("bod", [D, 1], f32, kind="ExternalInput")
    outT = nc.dram_tensor("outT", [D, SQ], f32, kind="ExternalOutput")

    with tile.TileContext(nc) as tc:
        with ExitStack() as ctx:
            ep = ctx.enter_context
            consts = ep(tc.tile_pool(name="consts", bufs=1))
            raw = ep(tc.tile_pool(name="raw", bufs=2))
            projq = ep(tc.tile_pool(name="projq", bufs=2))
            projk = ep(tc.tile_pool(name="projk", bufs=2))
            projv = ep(tc.tile_pool(name="projv", bufs=2))
            attn_p = ep(tc.tile_pool(name="attn", bufs=2 * (AV_LAG + 1)))
            norm_p = ep(tc.tile_pool(name="norm", bufs=2))
            hid_p = ep(tc.tile_pool(name="hid", bufs=1))
            outs_p = ep(tc.tile_pool(name="outs", bufs=2))
            sc_ps = ep(tc.tile_pool(name="scps", bufs=2, space="PSUM"))
            av_ps = ep(tc.tile_pool(name="avps", bufs=2, space="PSUM"))

            def mm512(out, lhsT, rhs, start=True, stop=True):
                n = out.shape[-1]
                assert rhs.shape[-1] == n
                for j in range(0, n, 512):
                    w = min(512, n - j)
                    nc.tensor.matmul(out[..., j:j + w], lhsT, rhs[..., j:j + w],
                                     start=start, stop=stop)

            # ---- small constants first (so warm-up + projections can start
            # while the big woT DMA streams in) ----
            wqA_s = consts.tile([128, 128], bf16, tag="wqA")
            nc.sync.dma_start(wqA_s[:], wqA[:, :])
            wqB_s = consts.tile([128, 128], bf16, tag="wqB")
            nc.sync.dma_start(wqB_s[:], wqB[:, :])
            wk2_s = consts.tile([128, 128], bf16, tag="wk2")
            nc.sync.dma_start(wk2_s[:], wk2[:, :])
            wv2_s = consts.tile([128, 128], bf16, tag="wv2")
            nc.sync.dma_start(wv2_s[:], wv2[:, :])
            bqA_s = consts.tile([128, 1], f32, tag="bqA")
            nc.sync.dma_start(bqA_s[:], bqA2[:, :])
            bqB_s = consts.tile([128, 1], f32, tag="bqB")
            nc.sync.dma_start(bqB_s[:], bqB2[:, :])
            bk2_s = consts.tile([128, 1], f32, tag="bk2")
            nc.sync.dma_start(bk2_s[:], bk2[:, :])
            c3t = consts.tile([128, 1], f32, tag="c3t")
            nc.vector.memset(c3t[:], EXPC0)

            hidden = hid_p.tile([128, 8, SQ], bf16, tag="hidden")

            # ---- PE warm-up: >3.4us of back-to-back matmuls flips the HAM
            # clock gate to 8/8 (2.4 GHz) before real work arrives.  Runs on
            # the small weight tiles while the pair-0 DMAs stream.
            warm = sc_ps.tile([128, SQ], f32, tag="sc")
            for i in range(36):
                nc.tensor.matmul(warm[:, 0:128], wqA_s[:], wk2_s[:],
                                 start=(i == 0), stop=(i == 35))

            # ---- per-pair prep stages, hoisted into the PREVIOUS pair's
            # chunk loop so the pair boundary has no serialized engine chain
            # (an idle PE window at the boundary re-throttles HAM for ~10us).
            def prep_raw(pair):
                rows = slice(pair * 128, (pair + 1) * 128)
                q2 = raw.tile([128, SQ], bf16, tag="q2")
                nc.sync.dma_start(q2[:], qT[rows, :])
                k2 = raw.tile([128, S], bf16, tag="k2")
                nc.sync.dma_start(k2[:], kT[rows, :])
                v2 = raw.tile([128, S], bf16, tag="v2")
                nc.sync.dma_start(v2[:], vT[rows, :])
                return q2, k2, v2

            def prep_qk(q2, k2):
                # Q projection, zero-padded per head: qhA rows 0:64 = head A,
                # rows 64:128 = exact zeros (zero weight block); qhB vice
                # versa.  Scores then contract the FULL 128 partitions of kh,
                # so every matmul in the kernel runs in 128-row tile mode --
                # no PE tile-mode switches (each switch drains the array and
                # the micro-bubbles pin the HAM clock gate at 4/8).
                qhA = projq.tile([128, SQ], bf16, tag="qhA")
                ps = sc_ps.tile([128, SQ], f32, tag="sc")
                mm512(ps[:], wqA_s[:], q2[:])
                nc.scalar.activation(qhA[:], ps[:], Ident, bias=bqA_s[:])
                qhB = projq.tile([128, SQ], bf16, tag="qhB")
                ps = sc_ps.tile([128, SQ], f32, tag="sc")
                mm512(ps[:], wqB_s[:], q2[:])
                nc.scalar.activation(qhB[:], ps[:], Ident, bias=bqB_s[:])
                kh = projk.tile([128, S], bf16, tag="kh")
                for half in range(2):
                    ps = sc_ps.tile([128, SQ], f32, tag="sc")
                    mm512(ps[:], wk2_s[:],
                          k2[:, half * 1024:(half + 1) * 1024])
                    nc.scalar.activation(
                        kh[:, half * 1024:(half + 1) * 1024], ps[:], Ident,
                        bias=bk2_s[:])
                return qhA, qhB, kh

            def prep_v(v2):
                # V projection (no bias -- folded into out-proj bias).
                # 4 chunks share one PSUM bank-group; one strided VectorE
                # copy moves them into the vha layout; ones columns memset.
                vha = projv.tile([128, NKC, VAW], bf16, tag="vha")
                nc.vector.memset(vha[:, :, 64:65], 1.0)
                nc.vector.memset(vha[:, :, 129:130], 1.0)
                for g in range(NKC // 4):
                    psv = sc_ps.tile([128, 512], f32, tag="sc")
                    for c in range(4):
                        sc_i = g * 4 + c
                        nc.tensor.matmul(
                            psv[:, c * 128:(c + 1) * 128],
                            v2[:, sc_i * 128:(sc_i + 1) * 128], wv2_s[:],
                            start=True, stop=True)
                    src = psv[:].rearrange("p (c h e) -> p c h e", c=4, e=64)
                    dst = vha[:, g * 4:(g + 1) * 4, :].rearrange(
                        "p c (h e) -> p c h e", e=65)[:, :, :, 0:64]
                    nc.vector.tensor_copy(dst, src)
                return vha

            rawt = {0: prep_raw(0)}
            woT_s = consts.tile([128, 8, D], bf16, tag="woT")
            nc.sync.dma_start(woT_s[:], woT.rearrange("(et p) o -> p et o", p=128))
            bo_s = consts.tile([128, 8], f32, tag="bo")
            nc.sync.dma_start(bo_s[:], bod.rearrange("(ot p) one -> p (ot one)", p=128))

            prepped = {0: prep_qk(rawt[0][0], rawt[0][1]) + (prep_v(rawt[0][2]),)}

            for pair in range(NPAIR):
                qhA, qhB, kh, vha = prepped.pop(pair)
                avA = av_ps.tile([65, SQ], f32, tag="av")
                avB = av_ps.tile([65, SQ], f32, tag="av")
                ats = {}

                def do_av(kc):
                    first, last = kc == 0, kc == NKC - 1
                    atA, atB = ats.pop(kc)
                    mm512(avA[:], vha[:, kc, 0:65], atA[:],
                          start=first, stop=last)
                    mm512(avB[:], vha[:, kc, 65:130], atB[:],
                          start=first, stop=last)

                nxt = pair + 1
                for kc in range(NKC):
                    ks = slice(kc * 128, (kc + 1) * 128)
                    scA = sc_ps.tile([128, SQ], f32, tag="sc")
                    scB = sc_ps.tile([128, SQ], f32, tag="sc")
                    for j in (0, 512):
                        nc.tensor.matmul(scA[:, j:j + 512], kh[:, ks],
                                         qhA[:, j:j + 512],
                                         start=True, stop=True)
                        nc.tensor.matmul(scB[:, j:j + 512], kh[:, ks],
                                         qhB[:, j:j + 512],
                                         start=True, stop=True)
                    # exp in 512-col halves, each tile split across BOTH
                    # engines (halves the tile's ready-latency); subtile deps
                    # let the next chunk's scores overwrite each half as soon
                    # as it has been read.
                    atA = attn_p.tile([128, SQ], bf16, tag="attn")
                    nc.scalar.activation(atA[:, 0:512], scA[:, 0:512], Exp,
                                         scale=0.125)
                    nc.vector._custom_dve(
                        exp_op, out=atA[:, 512:1024], in0=scA[:, 512:1024],
                        in1=c3t[:], s0=EXPC3, s1=EXPC2, imm2=EXPC1)
                    atB = attn_p.tile([128, SQ], bf16, tag="attn")
                    nc.vector._custom_dve(
                        exp_op, out=atB[:, 0:512], in0=scB[:, 0:512],
                        in1=c3t[:], s0=EXPC3, s1=EXPC2, imm2=EXPC1)
                    nc.scalar.activation(atB[:, 512:1024], scB[:, 512:1024],
                                         Exp, scale=0.125)
                    ats[kc] = (atA, atB)
                    if kc >= AV_LAG:
                        do_av(kc - AV_LAG)
                    if nxt < NPAIR:
                        if kc == 3:
                            rawt[nxt] = prep_raw(nxt)
                        elif kc == 8:
                            prepped[nxt] = prep_qk(rawt[nxt][0], rawt[nxt][1])
                        elif kc == 11:
                            prepped[nxt] = prepped[nxt] + (
                                prep_v(rawt.pop(nxt)[2]),)
                for kc in range(NKC - AV_LAG, NKC):
                    do_av(kc)

                # ---- normalize: hidden^T[e, q] = av[e, q] / av[64, q] ----
                # Whole-accumulator copies to SBUF (avsA on ScalarE, avsB on
                # VectorE) free the PSUM banks ~1us after the last matmul;
                # the reciprocal / broadcast / multiply chain runs off-PSUM
                # on VectorE+GpSimdE.
                for half, av in ((0, avA), (1, avB)):
                    avs = norm_p.tile([65, SQ], f32, tag="avs")
                    if half == 0:
                        nc.scalar.copy(avs[:], av[:])
                    else:
                        nc.vector.tensor_copy(avs[:], av[:])
                    sums = norm_p.tile([1, SQ], f32, tag="sums")
                    nc.sync.dma_start(sums[:], avs[64:65, :])
                    recip = norm_p.tile([1, SQ], f32, tag="recip")
                    nc.vector.reciprocal_approx_fast(recip[:], sums[:])
                    fac = norm_p.tile([64, SQ], f32, tag="fac")
                    nc.gpsimd.partition_broadcast(fac[:], recip[:])
                    if half == 0:
                        nc.gpsimd.tensor_tensor(
                            hidden[0:64, pair, :], avs[0:64, :], fac[:],
                            op=mybir.AluOpType.mult)
                    else:
                        stg = norm_p.tile([64, SQ], bf16, tag="stg")
                        nc.gpsimd.tensor_tensor(
                            stg[:], avs[0:64, :], fac[:],
                            op=mybir.AluOpType.mult)
                        nc.sync.dma_start(hidden[64:128, pair, :], stg[:])

            # ---- output projection: out^T[o, q] ----
            # e-tile 7 (the final pair) accumulates LAST so the projection
            # overlaps the final pair's normalize chain.
            et_order = list(range(7)) + [7]
            for ot in range(8):
                pso = sc_ps.tile([128, SQ], f32, tag="sc")
                for i, et in enumerate(et_order):
                    mm512(pso[:],
                          woT_s[:, et, ot * 128:(ot + 1) * 128],
                          hidden[:, et, :],
                          start=(i == 0), stop=(i == 7))
                o_s = outs_p.tile([128, SQ], f32, tag="outs")
                nc.scalar.activation(o_s[:], pso[:], Ident,
                                     bias=bo_s[:, ot:ot + 1])
                nc.sync.dma_start(outT[ot * 128:(ot + 1) * 128, :], o_s[:])

    nc.compile()
    return nc


def _get_nc():
    if "nc" not in _cache:
        _cache["nc"] = _build_program()
    return _cache["nc"]


def _prep_consts(Wq, bq, Wk, bk, Wv, bv, Wo, bo):
    f = np.float32

    def blockdiag2(W):
        out = np.zeros((128, 128), f)
        out[:64, :64] = W.T
        out[64:, 64:] = W.T
        return out

    import ml_dtypes
    b16 = ml_dtypes.bfloat16
    # V bias folded into the output-projection bias: hidden rows carry
    # (attn @ Wv x)/Z only, and out = hidden @ Wo^T + (bo + Wo @ tile(bv, H)).
    bo_fold = bo.astype(f) + Wo.astype(f) @ np.tile(bv.astype(f), H)
    wqA = np.zeros((128, 128), f)
    wqA[:64, :64] = Wq.T
    wqB = np.zeros((128, 128), f)
    wqB[64:, 64:] = Wq.T
    bqA2 = np.zeros((128, 1), f)
    bqA2[:64, 0] = bq
    bqB2 = np.zeros((128, 1), f)
    bqB2[64:, 0] = bq
    return {
        "wqA": wqA.astype(b16),
        "wqB": wqB.astype(b16),
        "wk2": blockdiag2(Wk).astype(b16),
        "wv2": blockdiag2(Wv).astype(b16),
        "bqA2": bqA2,
        "bqB2": bqB2,
        "bk2": np.tile(bk.astype(f), 2)[:, None].copy(),
        "woT": np.ascontiguousarray(Wo.T.astype(f)).astype(b16),
        "bod": bo_fold[:, None].copy(),
    }


def kernel(q, k, v, Wq, bq, Wk, bk, Wv, bv, Wo, bo, _trace=False):
    import ml_dtypes
    b16 = ml_dtypes.bfloat16
    q = np.asarray(q, np.float32)
    k = np.asarray(k, np.float32)
    v = np.asarray(v, np.float32)
    consts = _prep_consts(
        np.asarray(Wq, np.float32), np.asarray(bq, np.float32),
        np.asarray(Wk, np.float32), np.asarray(bk, np.float32),
        np.asarray(Wv, np.float32), np.asarray(bv, np.float32),
        np.asarray(Wo, np.float32), np.asarray(bo, np.float32))

    in_maps = []
    for c in range(N_CORES):
        b, chunk = c // 2, c % 2
        m = dict(consts)
        m["qT"] = np.ascontiguousarray(
            q[b, chunk * SQ:(chunk + 1) * SQ, :].T).astype(b16)
        m["kT"] = np.ascontiguousarray(k[b].T).astype(b16)
        m["vT"] = np.ascontiguousarray(v[b].T).astype(b16)
        in_maps.append(m)

    nc = _get_nc()
    from concourse.bass_utils import run_bass_kernel_spmd
    res = run_bass_kernel_spmd(nc, in_maps, core_ids=list(range(N_CORES)),
                               trace=_trace)
    if _trace:
        kernel.last_results = res

    out = np.empty((B, S, D), np.float32)
    for c in range(N_CORES):
        b, chunk = c // 2, c % 2
        out[b, chunk * SQ:(chunk + 1) * SQ, :] = res.results[c]["outT"].T
    return out


# revision 17
# speedup vs baseline: 1.4991x; 1.0127x over previous
"""Multi-head attention Trainium2 kernel (8 NeuronCores, SPMD).

Problem: B=4, S=2048, D_MODEL=1024, H=16, DIM=64 (nn_MultiHeadAttn).
Sharding: core c handles (batch b = c//2, query-row chunk c%2 of 1024).
Each core computes all 16 heads for its 1024 query rows against the full
2048 keys of its batch, then its rows of the output projection.

v3 — restructured to keep the PE HAM clock gate at 8/8 (2.4 GHz):
  - EVERY matmul runs in 128-row tile mode.  The Q projection produces
    zero-padded per-head tiles (qhA rows 64:128 are exact zeros from a zero
    weight block, and vice versa), so scores contract the full 128-partition
    kh tile.  Mixing 64-row scores with 128-row attn@V (v2) forced a PE
    tile-mode switch every chunk; each switch drains the systolic array and
    the resulting micro-bubbles keep the HAM activity monitor demoting the
    clock to 4/8 (measured: 86% of the run at 1.2 GHz).
  - attn@V runs 2 chunks behind the scores stream so the exp engines
    (ScalarE spline exp + VectorE custom poly exp) always have slack and
    the PE never micro-idles (micro-idles re-throttle HAM to 4/8).
  - The per-pair [65,SQ] PSUM accumulators (64 v-dims + sum-of-exp row) are
    copied whole to SBUF by ScalarE right after the last accumulation,
    freeing PSUM banks in ~1us; softmax normalization (fast reciprocal,
    partition-broadcast, multiply) then runs on VectorE+GpSimdE from SBUF.
  - V bias is folded into the output-projection bias host-side
    (out = (attn@(Wv x))/Z @ Wo^T + [bo + Wo @ tile(bv,H)]), removing a
    per-chunk bias pass.
  - Output projection accumulates e-tiles 0..6 first and e-tile 7 (the last
    pair) last, so it starts while the final pair is still normalizing.
"""

import sys

if "/opt/trn_rl_repo" not in sys.path:
    sys.path.insert(0, "/opt/trn_rl_repo")

import numpy as np
from contextlib import ExitStack

N_CORES = 8
B, S, D = 4, 2048, 1024
H, DIM = 16, 64
SQ = 1024          # query rows per core
NPAIR = 8          # head pairs
NKC = S // 128     # key chunks of 128
VAW = 130          # vha width: (64 v-dims + ones) * 2 heads
AV_LAG = 3         # attn@V trails the scores stream by this many chunks

# deg-3 minimax fit of exp(x/32) on |x|<=20; kernel computes p(x)^4=exp(x/8).
EXPC3 = 4.98779571e-06
EXPC2 = 5.03750782e-04
EXPC1 = 3.13034249e-02
EXPC0 = 9.99313241e-01

_cache = {}


def _register_exp_op():
    """Register the custom DVE exp op (deg-3 Horner + 2 squarings, 8 ALU
    stages) in concourse's custom-DVE registry; the per-NEFF uop table is
    generated from dve_ops.OPS at compile time."""
    if "exp_op" in _cache:
        return _cache["exp_op"]
    from concourse import dve_ops
    from concourse.dve_spec import (
        Spec, Src0, C0, C1, C2, C3, sq, lower, _spill_c3_to_src1,
    )
    from concourse.dve_uop import DveOpSpec
    from concourse.dve_table_gen import dve_ver_for

    name = "EXP_POLY4_ANT"
    for op in dve_ops.OPS:
        if op.name == name:
            _cache["exp_op"] = op
            return op

    def _ref(in0, in1, s0, s1, imm2):
        p = ((s0 * in0 + s1) * in0 + imm2) * in0 + in1
        return (p * p) * (p * p)

    body = sq(sq(((C0 * Src0 + C1) * Src0 + C2) * Src0 + C3))
    spec = Spec(body=_spill_c3_to_src1(body), reference=_ref)
    dve_ops._SUB_OPCODE_FOR_NAME[name] = dve_ops._CUSTOM_DVE_ROW_BASE + len(dve_ops.OPS)
    shas = {}
    for ver in ("v3", "v4"):
        try:
            tmp = DveOpSpec(name=name, opcode=dve_ops.get_dve_sub_opcode(name),
                            uops=lower(spec, ver=ver), rd1_en=True)
            shas[ver] = tmp.sha(ver)
        except Exception:
            pass
    op = dve_ops.DveOp(name, spec, subdim=False, uops_sha=shas)
    dve_ops.OPS.append(op)
    dve_ops.CUSTOM_DVE_SPECS[name] = spec
    _cache["exp_op"] = op
    return op


def _build_program():
    from concourse import bacc, mybir, tile

    exp_op = _register_exp_op()

    f32 = mybir.dt.float32
    bf16 = mybir.dt.bfloat16
    Exp = mybir.ActivationFunctionType.Exp
    Ident = mybir.ActivationFunctionType.Identity

    nc = bacc.Bacc("TRN2", target_bir_lowering=False, debug=False)

    qT = nc.dram_tensor("qT", [D, SQ], bf16, kind="ExternalInput")
    kT = nc.dram_tensor("kT", [D, S], bf16, kind="ExternalInput")
    vT = nc.dram_tensor("vT", [D, S], bf16, kind="ExternalInput")
    wqA = nc.dram_tensor("wqA", [128, 128], bf16, kind="ExternalInput")
    wqB = nc.dram_tensor("wqB", [128, 128], bf16, kind="ExternalInput")
    wk2 = nc.dram_tensor("wk2", [128, 128], bf16, kind="ExternalInput")
    wv2 = nc.dram_tensor("wv2", [128, 128], bf16, kind="ExternalInput")
    bqA2 = nc.dram_tensor("bqA2", [128, 1], f32, kind="ExternalInput")
    bqB2 = nc.dram_tensor("bqB2", [128, 1], f32, kind="ExternalInput")
    bk2 = nc.dram_tensor("bk2", [128, 1], f32, kind="ExternalInput")
    woT = nc.dram_tensor("woT", [D, D], bf16, kind="ExternalInput")
    bod = nc.dram_tensor("bod", [D, 1], f32, kind="ExternalInput")
    outT = nc.dram_tensor("outT", [D, SQ], f32, kind="ExternalOutput")

    with tile.TileContext(nc) as tc:
        with ExitStack() as ctx:
            ep = ctx.enter_context
            consts = ep(tc.tile_pool(name="consts", bufs=1))
            raw = ep(tc.tile_pool(name="raw", bufs=2))
            projq = ep(tc.tile_pool(name="projq", bufs=2))
            projk = ep(tc.tile_pool(name="projk", bufs=2))
            projv = ep(tc.tile_pool(name="projv", bufs=2))
            attn_p = ep(tc.tile_pool(name="attn", bufs=2 * (AV_LAG + 1)))
            norm_p = ep(tc.tile_pool(name="norm", bufs=2))
            hid_p = ep(tc.tile_pool(name="hid", bufs=1))
            outs_p = ep(tc.tile_pool(name="outs", bufs=2))
            sc_ps = ep(tc.tile_pool(name="scps", bufs=2, space="PSUM"))
            av_ps = ep(tc.tile_pool(name="avps", bufs=2, space="PSUM"))

            def mm512(out, lhsT, rhs, start=True, stop=True):
                n = out.shape[-1]
                assert rhs.shape[-1] == n
                for j in range(0, n, 512):
                    w = min(512, n - j)
                    nc.tensor.matmul(out[..., j:j + w], lhsT, rhs[..., j:j + w],
                                     start=start, stop=stop)

            # ---- small constants first (so warm-up + projections can start
            # while the big woT DMA streams in) ----
            wqA_s = consts.tile([128, 128], bf16, tag="wqA")
            nc.sync.dma_start(wqA_s[:], wqA[:, :])
            wqB_s = consts.tile([128, 128], bf16, tag="wqB")
            nc.sync.dma_start(wqB_s[:], wqB[:, :])
            wk2_s = consts.tile([128, 128], bf16, tag="wk2")
            nc.sync.dma_start(wk2_s[:], wk2[:, :])
            wv2_s = consts.tile([128, 128], bf16, tag="wv2")
            nc.sync.dma_start(wv2_s[:], wv2[:, :])
            bqA_s = consts.tile([128, 1], f32, tag="bqA")
            nc.sync.dma_start(bqA_s[:], bqA2[:, :])
            bqB_s = consts.tile([128, 1], f32, tag="bqB")
            nc.sync.dma_start(bqB_s[:], bqB2[:, :])
            bk2_s = consts.tile([128, 1], f32, tag="bk2")
            nc.sync.dma_start(bk2_s[:], bk2[:, :])
            c3t = consts.tile([128, 1], f32, tag="c3t")
            nc.vector.memset(c3t[:], EXPC0)

            # one hidden tile per pair so the output projection's reads
            # depend only on that pair's normalize (whole-tile deps on a
            # single [128,8,SQ] tensor serialized out-proj behind pair 7).
            hidden_t = [hid_p.tile([128, SQ], bf16, tag=f"hid{p}",
                                   name=f"hidden{p}")
                        for p in range(NPAIR)]

            # ---- PE warm-up: >3.4us of back-to-back matmuls flips the HAM
            # clock gate to 8/8 (2.4 GHz) before real work arrives.  Runs on
            # the small weight tiles while the pair-0 DMAs stream.
            warm = sc_ps.tile([128, SQ], f32, tag="sc")
            for i in range(56):
                nc.tensor.matmul(warm[:, 0:128], wqA_s[:], wk2_s[:],
                                 start=(i == 0), stop=(i == 55))

            # ---- per-pair prep stages, hoisted into the PREVIOUS pair's
            # chunk loop so the pair boundary has no serialized engine chain
            # (an idle PE window at the boundary re-throttles HAM for ~10us).
            def prep_raw(pair):
                rows = slice(pair * 128, (pair + 1) * 128)
                q2 = raw.tile([128, SQ], bf16, tag="q2")
                nc.sync.dma_start(q2[:], qT[rows, :])
                k2 = raw.tile([128, S], bf16, tag="k2")
                nc.sync.dma_start(k2[:], kT[rows, :])
                v2 = raw.tile([128, S], bf16, tag="v2")
                nc.sync.dma_start(v2[:], vT[rows, :])
                return q2, k2, v2

            def prep_qk(q2, k2):
                # Q projection, zero-padded per head: qhA rows 0:64 = head A,
                # rows 64:128 = exact zeros (zero weight block); qhB vice
                # versa.  Scores then contract the FULL 128 partitions of kh,
                # so every matmul in the kernel runs in 128-row tile mode --
                # no PE tile-mode switches (each switch drains the array and
                # the micro-bubbles pin the HAM clock gate at 4/8).
                qhA = projq.tile([128, SQ], bf16, tag="qhA")
                ps = sc_ps.tile([128, SQ], f32, tag="sc")
                mm512(ps[:], wqA_s[:], q2[:])
                nc.scalar.activation(qhA[:], ps[:], Ident, bias=bqA_s[:])
                qhB = projq.tile([128, SQ], bf16, tag="qhB")
                ps = sc_ps.tile([128, SQ], f32, tag="sc")
                mm512(ps[:], wqB_s[:], q2[:])
                nc.scalar.activation(qhB[:], ps[:], Ident, bias=bqB_s[:])
                kh = projk.tile([128, S], bf16, tag="kh")
                for half in range(2):
                    ps = sc_ps.tile([128, SQ], f32, tag="sc")
                    mm512(ps[:], wk2_s[:],
                          k2[:, half * 1024:(half + 1) * 1024])
                    nc.scalar.activation(
                        kh[:, half * 1024:(half + 1) * 1024], ps[:], Ident,
                        bias=bk2_s[:])
                return qhA, qhB, kh

            def prep_v(v2):
                # V projection (no bias -- folded into out-proj bias).
                # 4 chunks share one PSUM bank-group; one strided VectorE
                # copy moves them into the vha layout; ones columns memset.
                vha = projv.tile([128, NKC, VAW], bf16, tag="vha")
                nc.vector.memset(vha[:, :, 64:65], 1.0)
                nc.vector.memset(vha[:, :, 129:130], 1.0)
                for g in range(NKC // 4):
                    psv = sc_ps.tile([128, 512], f32, tag="sc")
                    for c in range(4):
                        sc_i = g * 4 + c
                        nc.tensor.matmul(
                            psv[:, c * 128:(c + 1) * 128],
                            v2[:, sc_i * 128:(sc_i + 1) * 128], wv2_s[:],
                            start=True, stop=True)
                    src = psv[:].rearrange("p (c h e) -> p c h e", c=4, e=64)
                    dst = vha[:, g * 4:(g + 1) * 4, :].rearrange(
                        "p c (h e) -> p c h e", e=65)[:, :, :, 0:64]
                    nc.vector.tensor_copy(dst, src)
                return vha

            rawt = {0: prep_raw(0)}
            woT_s = consts.tile([128, 8, D], bf16, tag="woT")
            nc.sync.dma_start(woT_s[:], woT.rearrange("(et p) o -> p et o", p=128))
            bo_s = consts.tile([128, 8], f32, tag="bo")
            nc.sync.dma_start(bo_s[:], bod.rearrange("(ot p) one -> p (ot one)", p=128))

            prepped = {0: prep_qk(rawt[0][0], rawt[0][1]) + (prep_v(rawt[0][2]),)}

            for pair in range(NPAIR):
                qhA, qhB, kh, vha = prepped.pop(pair)
                avA = av_ps.tile([65, SQ], f32, tag="av")
                avB = av_ps.tile([65, SQ], f32, tag="av")
                ats = {}

                def do_av(kc):
                    first, last = kc == 0, kc == NKC - 1
                    atA, atB = ats.pop(kc)
                    mm512(avA[:], vha[:, kc, 0:65], atA[:],
                          start=first, stop=last)
                    mm512(avB[:], vha[:, kc, 65:130], atB[:],
                          start=first, stop=last)

                nxt = pair + 1
                for kc in range(NKC):
                    ks = slice(kc * 128, (kc + 1) * 128)
                    scA = sc_ps.tile([128, SQ], f32, tag="sc")
                    scB = sc_ps.tile([128, SQ], f32, tag="sc")
                    for j in (0, 512):
                        nc.tensor.matmul(scA[:, j:j + 512], kh[:, ks],
                                         qhA[:, j:j + 512],
                                         start=True, stop=True)
                        nc.tensor.matmul(scB[:, j:j + 512], kh[:, ks],
                                         qhB[:, j:j + 512],
                                         start=True, stop=True)
                    # exp in 512-col halves, each tile split across BOTH
                    # engines (halves the tile's ready-latency); subtile deps
                    # let the next chunk's scores overwrite each half as soon
                    # as it has been read.
                    atA = attn_p.tile([128, SQ], bf16, tag="attn")
                    nc.scalar.activation(atA[:, 0:512], scA[:, 0:512], Exp,
                                         scale=0.125)
                    nc.vector._custom_dve(
                        exp_op, out=atA[:, 512:1024], in0=scA[:, 512:1024],
                        in1=c3t[:], s0=EXPC3, s1=EXPC2, imm2=EXPC1)
                    atB = attn_p.tile([128, SQ], bf16, tag="attn")
                    nc.vector._custom_dve(
                        exp_op, out=atB[:, 0:512], in0=scB[:, 0:512],
                        in1=c3t[:], s0=EXPC3, s1=EXPC2, imm2=EXPC1)
                    nc.scalar.activation(atB[:, 512:1024], scB[:, 512:1024],
                                         Exp, scale=0.125)
                    ats[kc] = (atA, atB)
                    if kc >= AV_LAG:
                        do_av(kc - AV_LAG)
                    if nxt < NPAIR:
                        if kc == 3:
                            rawt[nxt] = prep_raw(nxt)
                        elif kc == 8:
                            prepped[nxt] = prep_qk(rawt[nxt][0], rawt[nxt][1])
                        elif kc == 11:
                            prepped[nxt] = prepped[nxt] + (
                                prep_v(rawt.pop(nxt)[2]),)
                for kc in range(NKC - AV_LAG, NKC):
                    do_av(kc)

                # ---- normalize: hidden^T[e, q] = av[e, q] / av[64, q] ----
                # Whole-accumulator copies to SBUF (avsA on ScalarE, avsB on
                # VectorE) free the PSUM banks ~1us after the last matmul.
                # The reciprocal row is replicated to 64 partitions by a
                # stride-0-source DMA (the gpsimd partition_broadcast ucode
                # shares the engine with tensor_tensor and every library
                # switch costs ~6us of load + drain).  The multiplies run on
                # GpSimdE except for the final pair, where the exposed tail
                # runs on the then-idle VectorE instead.
                hid = hidden_t[pair]
                for half, av in ((0, avA), (1, avB)):
                    avs = norm_p.tile([65, SQ], f32, tag="avs")
                    if half == 0:
                        nc.scalar.copy(avs[:], av[:])
                    else:
                        nc.vector.tensor_copy(avs[:], av[:])
                    sums = norm_p.tile([1, SQ], f32, tag="sums")
                    nc.sync.dma_start(sums[:], avs[64:65, :])
                    recip = norm_p.tile([1, SQ], f32, tag="recip")
                    nc.vector.reciprocal_approx_fast(recip[:], sums[:])
                    fac = norm_p.tile([64, SQ], f32, tag="fac")
                    nc.sync.dma_start(
                        fac[:],
                        recip[0:1, :].unsqueeze(1).to_broadcast([1, 64, SQ]))
                    eng = nc.vector if pair == NPAIR - 1 else nc.gpsimd
                    if half == 0:
                        eng.tensor_tensor(
                            hid[0:64, :], avs[0:64, :], fac[:],
                            op=mybir.AluOpType.mult)
                    else:
                        stg = norm_p.tile([64, SQ], bf16, tag="stg")
                        eng.tensor_tensor(
                            stg[:], avs[0:64, :], fac[:],
                            op=mybir.AluOpType.mult)
                        nc.sync.dma_start(hid[64:128, :], stg[:])

            # ---- output projection: out^T[o, q] ----
            # e-tile 7 (the final pair) accumulates LAST so the projection
            # overlaps the final pair's normalize chain.
            et_order = list(range(7)) + [7]
            for ot in range(8):
                pso = sc_ps.tile([128, SQ], f32, tag="sc")
                for i, et in enumerate(et_order):
                    mm512(pso[:],
                          woT_s[:, et, ot * 128:(ot + 1) * 128],
                          hidden_t[et][:, :],
                          start=(i == 0), stop=(i == 7))
                o_s = outs_p.tile([128, SQ], f32, tag="outs")
                nc.scalar.activation(o_s[:], pso[:], Ident,
                                     bias=bo_s[:, ot:ot + 1])
                nc.sync.dma_start(outT[ot * 128:(ot + 1) * 128, :], o_s[:])

    nc.compile()
    return nc


def _get_nc():
    if "nc" not in _cache:
        _cache["nc"] = _build_program()
    return _cache["nc"]


def _prep_consts(Wq, bq, Wk, bk, Wv, bv, Wo, bo):
    f = np.float32

    def blockdiag2(W):
        out = np.zeros((128, 128), f)
        out[:64, :64] = W.T
        out[64:, 64:] = W.T
        return out

    import ml_dtypes
    b16 = ml_dtypes.bfloat16
    # V bias folded into the output-projection bias: hidden rows carry
    # (attn @ Wv x)/Z only, and out = hidden @ Wo^T + (bo + Wo @ tile(bv, H)).
    bo_fold = bo.astype(f) + Wo.astype(f) @ np.tile(bv.astype(f), H)
    wqA = np.zeros((128, 128), f)
    wqA[:64, :64] = Wq.T
    wqB = np.zeros((128, 128), f)
    wqB[64:, 64:] = Wq.T
    bqA2 = np.zeros((128, 1), f)
    bqA2[:64, 0] = bq
    bqB2 = np.zeros((128, 1), f)
    bqB2[64:, 0] = bq
    return {
        "wqA": wqA.astype(b16),
        "wqB": wqB.astype(b16),
        "wk2": blockdiag2(Wk).astype(b16),
        "wv2": blockdiag2(Wv).astype(b16),
        "bqA2": bqA2,
        "bqB2": bqB2,
        "bk2": np.tile(bk.astype(f), 2)[:, None].copy(),
        "woT": np.ascontiguousarray(Wo.T.astype(f)).astype(b16),
        "bod": bo_fold[:, None].copy(),
    }


def kernel(q, k, v, Wq, bq, Wk, bk, Wv, bv, Wo, bo, _trace=False):
    import ml_dtypes
    b16 = ml_dtypes.bfloat16
    q = np.asarray(q, np.float32)
    k = np.asarray(k, np.float32)
    v = np.asarray(v, np.float32)
    consts = _prep_consts(
        np.asarray(Wq, np.float32), np.asarray(bq, np.float32),
        np.asarray(Wk, np.float32), np.asarray(bk, np.float32),
        np.asarray(Wv, np.float32), np.asarray(bv, np.float32),
        np.asarray(Wo, np.float32), np.asarray(bo, np.float32))

    in_maps = []
    for c in range(N_CORES):
        b, chunk = c // 2, c % 2
        m = dict(consts)
        m["qT"] = np.ascontiguousarray(
            q[b, chunk * SQ:(chunk + 1) * SQ, :].T).astype(b16)
        m["kT"] = np.ascontiguousarray(k[b].T).astype(b16)
        m["vT"] = np.ascontiguousarray(v[b].T).astype(b16)
        in_maps.append(m)

    nc = _get_nc()
    from concourse.bass_utils import run_bass_kernel_spmd
    res = run_bass_kernel_spmd(nc, in_maps, core_ids=list(range(N_CORES)),
                               trace=_trace)
    if _trace:
        kernel.last_results = res

    out = np.empty((B, S, D), np.float32)
    for c in range(N_CORES):
        b, chunk = c // 2, c % 2
        out[b, chunk * SQ:(chunk + 1) * SQ, :] = res.results[c]["outT"].T
    return out


# revision 19
# speedup vs baseline: 1.5096x; 1.0070x over previous
"""Multi-head attention Trainium2 kernel (8 NeuronCores, SPMD).

Problem: B=4, S=2048, D_MODEL=1024, H=16, DIM=64 (nn_MultiHeadAttn).
Sharding: core c handles (batch b = c//2, query-row chunk c%2 of 1024).
Each core computes all 16 heads for its 1024 query rows against the full
2048 keys of its batch, then its rows of the output projection.

v3 — restructured to keep the PE HAM clock gate at 8/8 (2.4 GHz):
  - EVERY matmul runs in 128-row tile mode.  The Q projection produces
    zero-padded per-head tiles (qhA rows 64:128 are exact zeros from a zero
    weight block, and vice versa), so scores contract the full 128-partition
    kh tile.  Mixing 64-row scores with 128-row attn@V (v2) forced a PE
    tile-mode switch every chunk; each switch drains the systolic array and
    the resulting micro-bubbles keep the HAM activity monitor demoting the
    clock to 4/8 (measured: 86% of the run at 1.2 GHz).
  - attn@V runs 2 chunks behind the scores stream so the exp engines
    (ScalarE spline exp + VectorE custom poly exp) always have slack and
    the PE never micro-idles (micro-idles re-throttle HAM to 4/8).
  - The per-pair [65,SQ] PSUM accumulators (64 v-dims + sum-of-exp row) are
    copied whole to SBUF by ScalarE right after the last accumulation,
    freeing PSUM banks in ~1us; softmax normalization (fast reciprocal,
    partition-broadcast, multiply) then runs on VectorE+GpSimdE from SBUF.
  - V bias is folded into the output-projection bias host-side
    (out = (attn@(Wv x))/Z @ Wo^T + [bo + Wo @ tile(bv,H)]), removing a
    per-chunk bias pass.
  - Output projection accumulates e-tiles 0..6 first and e-tile 7 (the last
    pair) last, so it starts while the final pair is still normalizing.
"""

import sys

if "/opt/trn_rl_repo" not in sys.path:
    sys.path.insert(0, "/opt/trn_rl_repo")

import numpy as np
from contextlib import ExitStack

N_CORES = 8
B, S, D = 4, 2048, 1024
H, DIM = 16, 64
SQ = 1024          # query rows per core
NPAIR = 8          # head pairs
NKC = S // 128     # key chunks of 128
VAW = 130          # vha width: (64 v-dims + ones) * 2 heads
AV_LAG = 3         # attn@V trails the scores stream by this many chunks

# deg-3 minimax fit of exp(x/32) on |x|<=20; kernel computes p(x)^4=exp(x/8).
EXPC3 = 4.98779571e-06
EXPC2 = 5.03750782e-04
EXPC1 = 3.13034249e-02
EXPC0 = 9.99313241e-01

_cache = {}


def _register_exp_op():
    """Register the custom DVE exp op (deg-3 Horner + 2 squarings, 8 ALU
    stages) in concourse's custom-DVE registry; the per-NEFF uop table is
    generated from dve_ops.OPS at compile time."""
    if "exp_op" in _cache:
        return _cache["exp_op"]
    from concourse import dve_ops
    from concourse.dve_spec import (
        Spec, Src0, C0, C1, C2, C3, sq, lower, _spill_c3_to_src1,
    )
    from concourse.dve_uop import DveOpSpec
    from concourse.dve_table_gen import dve_ver_for

    name = "EXP_POLY4_ANT"
    for op in dve_ops.OPS:
        if op.name == name:
            _cache["exp_op"] = op
            return op

    def _ref(in0, in1, s0, s1, imm2):
        p = ((s0 * in0 + s1) * in0 + imm2) * in0 + in1
        return (p * p) * (p * p)

    body = sq(sq(((C0 * Src0 + C1) * Src0 + C2) * Src0 + C3))
    spec = Spec(body=_spill_c3_to_src1(body), reference=_ref)
    dve_ops._SUB_OPCODE_FOR_NAME[name] = dve_ops._CUSTOM_DVE_ROW_BASE + len(dve_ops.OPS)
    shas = {}
    for ver in ("v3", "v4"):
        try:
            tmp = DveOpSpec(name=name, opcode=dve_ops.get_dve_sub_opcode(name),
                            uops=lower(spec, ver=ver), rd1_en=True)
            shas[ver] = tmp.sha(ver)
        except Exception:
            pass
    op = dve_ops.DveOp(name, spec, subdim=False, uops_sha=shas)
    dve_ops.OPS.append(op)
    dve_ops.CUSTOM_DVE_SPECS[name] = spec
    _cache["exp_op"] = op
    return op


def _build_program():
    from concourse import bacc, mybir, tile

    exp_op = _register_exp_op()

    f32 = mybir.dt.float32
    bf16 = mybir.dt.bfloat16
    Exp = mybir.ActivationFunctionType.Exp
    Ident = mybir.ActivationFunctionType.Identity

    nc = bacc.Bacc("TRN2", target_bir_lowering=False, debug=False)

    qT = nc.dram_tensor("qT", [D, SQ], bf16, kind="ExternalInput")
    kT = nc.dram_tensor("kT", [D, S], bf16, kind="ExternalInput")
    vT = nc.dram_tensor("vT", [D, S], bf16, kind="ExternalInput")
    wqA = nc.dram_tensor("wqA", [128, 128], bf16, kind="ExternalInput")
    wqB = nc.dram_tensor("wqB", [128, 128], bf16, kind="ExternalInput")
    wk2 = nc.dram_tensor("wk2", [128, 128], bf16, kind="ExternalInput")
    wv2 = nc.dram_tensor("wv2", [128, 128], bf16, kind="ExternalInput")
    bqA2 = nc.dram_tensor("bqA2", [128, 1], f32, kind="ExternalInput")
    bqB2 = nc.dram_tensor("bqB2", [128, 1], f32, kind="ExternalInput")
    bk2 = nc.dram_tensor("bk2", [128, 1], f32, kind="ExternalInput")
    woT = nc.dram_tensor("woT", [D, D], bf16, kind="ExternalInput")
    bod = nc.dram_tensor("bod", [D, 1], f32, kind="ExternalInput")
    outT = nc.dram_tensor("outT", [D, SQ], f32, kind="ExternalOutput")

    with tile.TileContext(nc) as tc:
        with ExitStack() as ctx:
            ep = ctx.enter_context
            consts = ep(tc.tile_pool(name="consts", bufs=1))
            raw = ep(tc.tile_pool(name="raw", bufs=2))
            projq = ep(tc.tile_pool(name="projq", bufs=2))
            projk = ep(tc.tile_pool(name="projk", bufs=2))
            projv = ep(tc.tile_pool(name="projv", bufs=2))
            attn_p = ep(tc.tile_pool(name="attn", bufs=2 * (AV_LAG + 1)))
            norm_p = ep(tc.tile_pool(name="norm", bufs=2))
            hid_p = ep(tc.tile_pool(name="hid", bufs=1))
            outs_p = ep(tc.tile_pool(name="outs", bufs=2))
            sc_ps = ep(tc.tile_pool(name="scps", bufs=2, space="PSUM"))
            av_ps = ep(tc.tile_pool(name="avps", bufs=2, space="PSUM"))

            def mm512(out, lhsT, rhs, start=True, stop=True):
                n = out.shape[-1]
                assert rhs.shape[-1] == n
                for j in range(0, n, 512):
                    w = min(512, n - j)
                    nc.tensor.matmul(out[..., j:j + w], lhsT, rhs[..., j:j + w],
                                     start=start, stop=stop)

            # ---- small constants first (so warm-up + projections can start
            # while the big woT DMA streams in) ----
            wqA_s = consts.tile([128, 128], bf16, tag="wqA")
            nc.sync.dma_start(wqA_s[:], wqA[:, :])
            wqB_s = consts.tile([128, 128], bf16, tag="wqB")
            nc.sync.dma_start(wqB_s[:], wqB[:, :])
            wk2_s = consts.tile([128, 128], bf16, tag="wk2")
            nc.sync.dma_start(wk2_s[:], wk2[:, :])
            wv2_s = consts.tile([128, 128], bf16, tag="wv2")
            nc.sync.dma_start(wv2_s[:], wv2[:, :])
            bqA_s = consts.tile([128, 1], f32, tag="bqA")
            nc.sync.dma_start(bqA_s[:], bqA2[:, :])
            bqB_s = consts.tile([128, 1], f32, tag="bqB")
            nc.sync.dma_start(bqB_s[:], bqB2[:, :])
            bk2_s = consts.tile([128, 1], f32, tag="bk2")
            nc.sync.dma_start(bk2_s[:], bk2[:, :])
            c3t = consts.tile([128, 1], f32, tag="c3t")
            nc.vector.memset(c3t[:], EXPC0)

            # one hidden tile per pair so the output projection's reads
            # depend only on that pair's normalize (whole-tile deps on a
            # single [128,8,SQ] tensor serialized out-proj behind pair 7).
            hidden_t = [hid_p.tile([128, SQ], bf16, tag=f"hid{p}",
                                   name=f"hidden{p}")
                        for p in range(NPAIR)]

            # ---- PE warm-up: >3.4us of back-to-back matmuls flips the HAM
            # clock gate to 8/8 (2.4 GHz) before real work arrives.  Runs on
            # the small weight tiles while the pair-0 DMAs stream.
            wmA = consts.tile([128, 128], bf16, tag="wmA")
            nc.vector.memset(wmA[:], 0.01)
            wmB = consts.tile([128, 128], bf16, tag="wmB")
            nc.vector.memset(wmB[:], 0.01)
            warm = sc_ps.tile([128, SQ], f32, tag="sc")
            for i in range(56):
                nc.tensor.matmul(warm[:, 0:128], wmA[:], wmB[:],
                                 start=(i == 0), stop=(i == 55))

            # ---- per-pair prep stages, hoisted into the PREVIOUS pair's
            # chunk loop so the pair boundary has no serialized engine chain
            # (an idle PE window at the boundary re-throttles HAM for ~10us).
            def prep_raw(pair):
                rows = slice(pair * 128, (pair + 1) * 128)
                q2 = raw.tile([128, SQ], bf16, tag="q2")
                nc.sync.dma_start(q2[:], qT[rows, :])
                k2 = raw.tile([128, S], bf16, tag="k2")
                nc.sync.dma_start(k2[:], kT[rows, :])
                v2 = raw.tile([128, S], bf16, tag="v2")
                nc.sync.dma_start(v2[:], vT[rows, :])
                return q2, k2, v2

            def prep_qk(q2, k2):
                # Q projection, zero-padded per head: qhA rows 0:64 = head A,
                # rows 64:128 = exact zeros (zero weight block); qhB vice
                # versa.  Scores then contract the FULL 128 partitions of kh,
                # so every matmul in the kernel runs in 128-row tile mode --
                # no PE tile-mode switches (each switch drains the array and
                # the micro-bubbles pin the HAM clock gate at 4/8).
                qhA = projq.tile([128, SQ], bf16, tag="qhA")
                ps = sc_ps.tile([128, SQ], f32, tag="sc")
                mm512(ps[:], wqA_s[:], q2[:])
                nc.scalar.activation(qhA[:], ps[:], Ident, bias=bqA_s[:])
                qhB = projq.tile([128, SQ], bf16, tag="qhB")
                ps = sc_ps.tile([128, SQ], f32, tag="sc")
                mm512(ps[:], wqB_s[:], q2[:])
                nc.scalar.activation(qhB[:], ps[:], Ident, bias=bqB_s[:])
                kh = projk.tile([128, S], bf16, tag="kh")
                for half in range(2):
                    ps = sc_ps.tile([128, SQ], f32, tag="sc")
                    mm512(ps[:], wk2_s[:],
                          k2[:, half * 1024:(half + 1) * 1024])
                    nc.scalar.activation(
                        kh[:, half * 1024:(half + 1) * 1024], ps[:], Ident,
                        bias=bk2_s[:])
                return qhA, qhB, kh

            def prep_v(v2):
                # V projection (no bias -- folded into out-proj bias).
                # 4 chunks share one PSUM bank-group; one strided VectorE
                # copy moves them into the vha layout; ones columns memset.
                vha = projv.tile([128, NKC, VAW], bf16, tag="vha")
                nc.vector.memset(vha[:, :, 64:65], 1.0)
                nc.vector.memset(vha[:, :, 129:130], 1.0)
                for g in range(NKC // 4):
                    psv = sc_ps.tile([128, 512], f32, tag="sc")
                    for c in range(4):
                        sc_i = g * 4 + c
                        nc.tensor.matmul(
                            psv[:, c * 128:(c + 1) * 128],
                            v2[:, sc_i * 128:(sc_i + 1) * 128], wv2_s[:],
                            start=True, stop=True)
                    src = psv[:].rearrange("p (c h e) -> p c h e", c=4, e=64)
                    dst = vha[:, g * 4:(g + 1) * 4, :].rearrange(
                        "p c (h e) -> p c h e", e=65)[:, :, :, 0:64]
                    nc.vector.tensor_copy(dst, src)
                return vha

            rawt = {0: prep_raw(0)}
            woT_s = consts.tile([128, 8, D], bf16, tag="woT")
            nc.sync.dma_start(woT_s[:], woT.rearrange("(et p) o -> p et o", p=128))
            bo_s = consts.tile([128, 8], f32, tag="bo")
            nc.sync.dma_start(bo_s[:], bod.rearrange("(ot p) one -> p (ot one)", p=128))

            prepped = {0: prep_qk(rawt[0][0], rawt[0][1]) + (prep_v(rawt[0][2]),)}

            for pair in range(NPAIR):
                qhA, qhB, kh, vha = prepped.pop(pair)
                avA = av_ps.tile([65, SQ], f32, tag="av")
                avB = av_ps.tile([65, SQ], f32, tag="av")
                ats = {}

                def do_av(kc):
                    first, last = kc == 0, kc == NKC - 1
                    atA, atB = ats.pop(kc)
                    mm512(avA[:], vha[:, kc, 0:65], atA[:],
                          start=first, stop=last)
                    mm512(avB[:], vha[:, kc, 65:130], atB[:],
                          start=first, stop=last)

                nxt = pair + 1
                for kc in range(NKC):
                    ks = slice(kc * 128, (kc + 1) * 128)
                    scA = sc_ps.tile([128, SQ], f32, tag="sc")
                    scB = sc_ps.tile([128, SQ], f32, tag="sc")
                    for j in (0, 512):
                        nc.tensor.matmul(scA[:, j:j + 512], kh[:, ks],
                                         qhA[:, j:j + 512],
                                         start=True, stop=True)
                        nc.tensor.matmul(scB[:, j:j + 512], kh[:, ks],
                                         qhB[:, j:j + 512],
                                         start=True, stop=True)
                    # exp in 512-col halves, each tile split across BOTH
                    # engines (halves the tile's ready-latency); subtile deps
                    # let the next chunk's scores overwrite each half as soon
                    # as it has been read.
                    atA = attn_p.tile([128, SQ], bf16, tag="attn")
                    nc.scalar.activation(atA[:, 0:512], scA[:, 0:512], Exp,
                                         scale=0.125)
                    nc.vector._custom_dve(
                        exp_op, out=atA[:, 512:1024], in0=scA[:, 512:1024],
                        in1=c3t[:], s0=EXPC3, s1=EXPC2, imm2=EXPC1)
                    atB = attn_p.tile([128, SQ], bf16, tag="attn")
                    nc.vector._custom_dve(
                        exp_op, out=atB[:, 0:512], in0=scB[:, 0:512],
                        in1=c3t[:], s0=EXPC3, s1=EXPC2, imm2=EXPC1)
                    nc.scalar.activation(atB[:, 512:1024], scB[:, 512:1024],
                                         Exp, scale=0.125)
                    ats[kc] = (atA, atB)
                    if kc >= AV_LAG:
                        do_av(kc - AV_LAG)
                    if nxt < NPAIR:
                        if kc == 3:
                            rawt[nxt] = prep_raw(nxt)
                        elif kc == 8:
                            prepped[nxt] = prep_qk(rawt[nxt][0], rawt[nxt][1])
                        elif kc == 11:
                            prepped[nxt] = prepped[nxt] + (
                                prep_v(rawt.pop(nxt)[2]),)
                    elif kc == 2:
                        # preload the gpsimd partition_broadcast library
                        # mid-final-pair (a library switch costs ~6us; this
                        # hides it so the tail broadcasts run immediately)
                        dmy = norm_p.tile([64, 1], f32, tag="dmy")
                        nc.gpsimd.partition_broadcast(dmy[:], c3t[0:1, :])
                for kc in range(NKC - AV_LAG, NKC):
                    do_av(kc)

                # ---- normalize: hidden^T[e, q] = av[e, q] / av[64, q] ----
                # Whole-accumulator copies to SBUF (avsA on ScalarE, avsB on
                # VectorE) free the PSUM banks ~1us after the last matmul.
                # The reciprocal row is replicated to 64 partitions by a
                # stride-0-source DMA (the gpsimd partition_broadcast ucode
                # shares the engine with tensor_tensor and every library
                # switch costs ~6us of load + drain).  The multiplies run on
                # GpSimdE except for the final pair, where the exposed tail
                # runs on the then-idle VectorE instead.
                hid = hidden_t[pair]
                for half, av in ((0, avA), (1, avB)):
                    avs = norm_p.tile([65, SQ], f32, tag="avs")
                    if half == 0:
                        nc.scalar.copy(avs[:], av[:])
                    else:
                        nc.vector.tensor_copy(avs[:], av[:])
                    sums = norm_p.tile([1, SQ], f32, tag="sums")
                    nc.sync.dma_start(sums[:], avs[64:65, :])
                    recip = norm_p.tile([1, SQ], f32, tag="recip")
                    nc.vector.reciprocal_approx_fast(recip[:], sums[:])
                    fac = norm_p.tile([64, SQ], f32, tag="fac")
                    if pair == NPAIR - 1:
                        # replication DMA is ~10us (64 stride-0 rows); on the
                        # exposed tail use the gpsimd broadcast ucode instead
                        nc.gpsimd.partition_broadcast(fac[:], recip[:])
                        eng = nc.vector
                    else:
                        nc.sync.dma_start(
                            fac[:],
                            recip[0:1, :].unsqueeze(1).to_broadcast(
                                [1, 64, SQ]))
                        eng = nc.gpsimd
                    if half == 0:
                        eng.tensor_tensor(
                            hid[0:64, :], avs[0:64, :], fac[:],
                            op=mybir.AluOpType.mult)
                    else:
                        stg = norm_p.tile([64, SQ], bf16, tag="stg")
                        eng.tensor_tensor(
                            stg[:], avs[0:64, :], fac[:],
                            op=mybir.AluOpType.mult)
                        nc.sync.dma_start(hid[64:128, :], stg[:])

            # ---- output projection: out^T[o, q] ----
            # e-tile 7 (the final pair) accumulates LAST so the projection
            # overlaps the final pair's normalize chain.
            et_order = list(range(7)) + [7]
            for ot in range(8):
                pso = av_ps.tile([128, SQ], f32, tag="av")
                for i, et in enumerate(et_order):
                    mm512(pso[:],
                          woT_s[:, et, ot * 128:(ot + 1) * 128],
                          hidden_t[et][:, :],
                          start=(i == 0), stop=(i == 7))
                o_s = outs_p.tile([128, SQ], f32, tag="outs")
                nc.scalar.activation(o_s[:], pso[:], Ident,
                                     bias=bo_s[:, ot:ot + 1])
                nc.sync.dma_start(outT[ot * 128:(ot + 1) * 128, :], o_s[:])

    nc.compile()
    return nc


def _get_nc():
    if "nc" not in _cache:
        _cache["nc"] = _build_program()
    return _cache["nc"]


def _prep_consts(Wq, bq, Wk, bk, Wv, bv, Wo, bo):
    f = np.float32

    def blockdiag2(W):
        out = np.zeros((128, 128), f)
        out[:64, :64] = W.T
        out[64:, 64:] = W.T
        return out

    import ml_dtypes
    b16 = ml_dtypes.bfloat16
    # V bias folded into the output-projection bias: hidden rows carry
    # (attn @ Wv x)/Z only, and out = hidden @ Wo^T + (bo + Wo @ tile(bv, H)).
    bo_fold = bo.astype(f) + Wo.astype(f) @ np.tile(bv.astype(f), H)
    wqA = np.zeros((128, 128), f)
    wqA[:64, :64] = Wq.T
    wqB = np.zeros((128, 128), f)
    wqB[64:, 64:] = Wq.T
    bqA2 = np.zeros((128, 1), f)
    bqA2[:64, 0] = bq
    bqB2 = np.zeros((128, 1), f)
    bqB2[64:, 0] = bq
    return {
        "wqA": wqA.astype(b16),
        "wqB": wqB.astype(b16),
        "wk2": blockdiag2(Wk).astype(b16),
        "wv2": blockdiag2(Wv).astype(b16),
        "bqA2": bqA2,
        "bqB2": bqB2,
        "bk2": np.tile(bk.astype(f), 2)[:, None].copy(),
        "woT": np.ascontiguousarray(Wo.T.astype(f)).astype(b16),
        "bod": bo_fold[:, None].copy(),
    }


def kernel(q, k, v, Wq, bq, Wk, bk, Wv, bv, Wo, bo, _trace=False):
    import ml_dtypes
    b16 = ml_dtypes.bfloat16
    q = np.asarray(q, np.float32)
    k = np.asarray(k, np.float32)
    v = np.asarray(v, np.float32)
    consts = _prep_consts(
        np.asarray(Wq, np.float32), np.asarray(bq, np.float32),
        np.asarray(Wk, np.float32), np.asarray(bk, np.float32),
        np.asarray(Wv, np.float32), np.asarray(bv, np.float32),
        np.asarray(Wo, np.float32), np.asarray(bo, np.float32))

    in_maps = []
    for c in range(N_CORES):
        b, chunk = c // 2, c % 2
        m = dict(consts)
        m["qT"] = np.ascontiguousarray(
            q[b, chunk * SQ:(chunk + 1) * SQ, :].T).astype(b16)
        m["kT"] = np.ascontiguousarray(k[b].T).astype(b16)
        m["vT"] = np.ascontiguousarray(v[b].T).astype(b16)
        in_maps.append(m)

    nc = _get_nc()
    from concourse.bass_utils import run_bass_kernel_spmd
    res = run_bass_kernel_spmd(nc, in_maps, core_ids=list(range(N_CORES)),
                               trace=_trace)
    if _trace:
        kernel.last_results = res

    out = np.empty((B, S, D), np.float32)
    for c in range(N_CORES):
        b, chunk = c // 2, c % 2
        out[b, chunk * SQ:(chunk + 1) * SQ, :] = res.results[c]["outT"].T
    return out
